# revision 7
# baseline (speedup 1.0000x reference)
"""Trainium2 Bass kernel for nn_CaptionNet_23467701305971.

Model: image-captioning net. init MLPs -> 2-layer biLSTM with a redundant
prefix-recomputation state chain (50 sequential calls, 275 LSTM steps per
direction-chain) -> big FC head to vocab 30000.

Strategy (8 NeuronCores):
  - The 4 direction-chains (l0f, l0b, l1f, l1b) are strictly sequential
    inside, but l0f/l0b are independent and l1f/l1b depend on l0 outputs.
  - Phase 1: chain NEFF (one SPMD program, role differences are pure DATA):
    core 0 runs the layer-0 forward chain, core 1 the layer-0 backward chain
    (backward = same program on time-reversed per-call inputs).
  - Host glue: assemble layer-1 inputs x1 = concat(of, ob) in consumption
    order per direction.
  - Phase 2: same chain program (wider input dim) runs layer-1 fwd/bwd on
    cores 0/1.
  - Phase 3: FC head, vocab-sharded across all 8 cores.
  - All matmuls bf16 with fp32 PSUM accumulation; cell state c and gate
    pre-activations stay fp32.  Measured numeric error vs the fp32
    reference: ~4e-3 relative L2.

Kernel layout notes:
  - Everything is "transposed": H lives on SBUF partitions. The recurrent
    matmul is weight-stationary: 64 (LDW+MM) pairs of [128k x 128m] @ [128k,
    16batch] per step, gates land on partitions so sigmoid/tanh run on 128
    lanes.
  - Gate order is host-permuted to (i, f, o, g) so one ACT op covers all
    sigmoids.
  - The per-call input projection (xg = x @ Wih + b) is emitted interleaved
    with chain steps two calls ahead, filling PE bubbles left by the
    elementwise chain.
"""

import os
import sys
import numpy as np
import ml_dtypes

sys.path.insert(0, "/opt/trn_rl_repo")

import concourse.bass as bass  # noqa: E402
from concourse import bacc  # noqa: E402
import concourse.tile as tile  # noqa: E402
import concourse.mybir as mybir  # noqa: E402

BF16 = mybir.dt.bfloat16
F32 = mybir.dt.float32
AF = mybir.ActivationFunctionType
ALU = mybir.AluOpType

B, N, T, H, E, V, F = 16, 5, 10, 512, 250, 30000, 2048
CALLS = [(t, n) for t in range(T) for n in range(N)]
LS = [t + 1 for (t, n) in CALLS]
POS0 = np.concatenate([[0], np.cumsum(LS)]).astype(int)
NPOS = int(POS0[-1])  # 275
NCORES = 8
VL = V // NCORES  # 3750
RPAD = 896  # 800 output rows padded to 7*128

nbf = ml_dtypes.bfloat16


# ---------------------------------------------------------------- host prep

def _perm_gates(W):
    """reorder gate blocks (i,f,g,o) -> (i,f,o,g) along the last axis."""
    Hh = W.shape[-1] // 4
    return np.concatenate(
        [W[..., :Hh], W[..., Hh:2 * Hh], W[..., 3 * Hh:], W[..., 2 * Hh:3 * Hh]],
        axis=-1)


def _tile_w(W, KX, MT):
    """[Din, MT*128] -> [128, KX, MT, 128] bf16 stationary tiles."""
    Din, M = W.shape
    assert M == MT * 128
    Wp = np.zeros((KX * 128, M), np.float32)
    Wp[:Din] = W
    return np.ascontiguousarray(
        Wp.reshape(KX, 128, MT, 128).transpose(1, 0, 2, 3)).astype(nbf)


def _tile_b(b, MT):
    return np.ascontiguousarray(b.reshape(MT, 128).T).astype(np.float32)


def _chain_host_inputs(inp):
    """Per-core input dicts for the two chain phases (minus the x inputs)."""
    com = {
        "imgT": np.ascontiguousarray(
            inp["img"].T.reshape(16, 128, B).transpose(1, 0, 2)).astype(nbf),
        "Wh1t": _tile_w(inp["Wh1"], 16, 8), "bh1t": _tile_b(inp["bh1"], 8),
        "Wh2t": _tile_w(inp["Wh2"], 8, 4), "bh2t": _tile_b(inp["bh2"], 4),
        "Wc1t": _tile_w(inp["Wc1"], 16, 8), "bc1t": _tile_b(inp["bc1"], 8),
        "Wc2t": _tile_w(inp["Wc2"], 8, 4), "bc2t": _tile_b(inp["bc2"], 4),
    }
    per_dir = {}
    for d, sfx in ((0, "f"), (1, "b")):
        per_dir[d] = dict(com)
    for d, sfx in ((0, "f"), (1, "b")):
        per_dir[d]["Whh0"] = _tile_w(_perm_gates(inp["Whh0" + sfx]), 4, 16)
        per_dir[d]["Wih0"] = _tile_w(_perm_gates(inp["Wih0" + sfx]), 2, 16)
        per_dir[d]["bg0"] = _tile_b(_perm_gates(inp["b0" + sfx]), 16)
        per_dir[d]["Whh1"] = _tile_w(_perm_gates(inp["Whh1" + sfx]), 4, 16)
        per_dir[d]["Wih1"] = _tile_w(_perm_gates(inp["Wih1" + sfx]), 8, 16)
        per_dir[d]["bg1"] = _tile_b(_perm_gates(inp["b1" + sfx]), 16)
    return per_dir


def _x0_arranged(inp, rev):
    """layer-0 chain input, consumption order, transposed: [128, 2, NPOS*B]."""
    seq = inp["emb"][inp["caps"]].transpose(1, 2, 0, 3)  # [N, T, B, E]
    A = np.zeros((NPOS, B, 256), np.float32)
    for k, (t, n) in enumerate(CALLS):
        L = t + 1
        for s in range(L):
            tok = (L - 1 - s) if rev else s
            A[POS0[k] + s] = np.pad(seq[n, tok], ((0, 0), (0, 6)))
    return np.ascontiguousarray(
        A.reshape(NPOS * B, 2, 128).transpose(2, 1, 0)).astype(nbf)


def _oh_to_HposB(oh):
    """device oh [128, 4, NPOS, B] -> [H, NPOS, B] float32."""
    return oh.astype(np.float32).transpose(1, 0, 2, 3).reshape(H, NPOS, B)


def _x1_arranged(hf, hb, rev):
    """layer-1 chain input [128, 8, NPOS*B] bf16 from layer-0 outputs.

    hf/hb: [H, NPOS, B] layer-0 fwd/bwd chain outputs in their own
    consumption order (fwd slot s = natural s; bwd slot s = natural L-1-s).
    """
    pf = np.zeros(NPOS, int)
    pb = np.zeros(NPOS, int)
    for k, (t, n) in enumerate(CALLS):
        L = t + 1
        for s in range(L):  # s = consumption slot of the l1 chain
            nat = (L - 1 - s) if rev else s  # natural time of this slot
            pf[POS0[k] + s] = POS0[k] + nat          # fwd chain slot = nat
            pb[POS0[k] + s] = POS0[k] + (L - 1 - nat)  # bwd chain slot
    A = np.concatenate([hf[:, pf, :], hb[:, pb, :]], axis=0)  # [1024, NPOS, B]
    return np.ascontiguousarray(
        A.reshape(8, 128, NPOS * B).transpose(1, 0, 2)).astype(nbf)


def _y_assemble(h1f, h1b):
    """final FC input yT [128, 8, RPAD] bf16 from layer-1 chain outputs."""
    y = np.zeros((2 * H, RPAD), np.float32)
    for n in range(N):
        k = 45 + n
        L = 10
        for s in range(L):
            r = (n * T + s) * B
            y[:H, r:r + B] = h1f[:, POS0[k] + s, :]
            y[H:, r:r + B] = h1b[:, POS0[k] + L - 1 - s, :]
    return np.ascontiguousarray(
        y.reshape(8, 128, RPAD).transpose(1, 0, 2)).astype(nbf)


# ---------------------------------------------------------------- builders

def build_chain(KX):
    """Chain NEFF. KX = input k-tiles (2 for layer-0, 8 for layer-1).

    Inputs (per core): imgT, W/b for both init MLPs, xt [128,KX,NPOS*B] bf16
    (arranged consumption-order rows, transposed), Wih [128,KX,16,128],
    bg [128,16], Whh [128,4,16,128].
    Output: oh [128, 4, NPOS, B] bf16 (per-slot hidden states).
    """
    nc = bacc.Bacc()
    imgT = nc.dram_tensor("imgT", [128, 16, B], BF16, kind="ExternalInput")
    Wh1 = nc.dram_tensor("Wh1t", [128, 16, 8, 128], BF16, kind="ExternalInput")
    bh1 = nc.dram_tensor("bh1t", [128, 8], F32, kind="ExternalInput")
    Wh2 = nc.dram_tensor("Wh2t", [128, 8, 4, 128], BF16, kind="ExternalInput")
    bh2 = nc.dram_tensor("bh2t", [128, 4], F32, kind="ExternalInput")
    Wc1 = nc.dram_tensor("Wc1t", [128, 16, 8, 128], BF16, kind="ExternalInput")
    bc1 = nc.dram_tensor("bc1t", [128, 8], F32, kind="ExternalInput")
    Wc2 = nc.dram_tensor("Wc2t", [128, 8, 4, 128], BF16, kind="ExternalInput")
    bc2 = nc.dram_tensor("bc2t", [128, 4], F32, kind="ExternalInput")
    xt = nc.dram_tensor("xt", [128, KX, NPOS * B], BF16, kind="ExternalInput")
    Wih = nc.dram_tensor("Wih", [128, KX, 16, 128], BF16, kind="ExternalInput")
    bg = nc.dram_tensor("bg", [128, 16], F32, kind="ExternalInput")
    Whh = nc.dram_tensor("Whh", [128, 4, 16, 128], BF16, kind="ExternalInput")
    oh = nc.dram_tensor("oh", [128, 4, NPOS, B], BF16, kind="ExternalOutput")

    with tile.TileContext(nc) as tc:
        with (
            tc.tile_pool(name="const", bufs=1) as cp,
            tc.tile_pool(name="xp", bufs=3) as xp,
            tc.tile_pool(name="xgp", bufs=3) as xgp,
            tc.tile_pool(name="hp", bufs=2) as hp,
            tc.tile_pool(name="ewp", bufs=2) as ewp,
            tc.tile_pool(name="sp", bufs=1) as sp,
            tc.tile_pool(name="pgp", bufs=2, space="PSUM") as pgp,
            tc.tile_pool(name="ppp", bufs=2, space="PSUM") as ppp,
            tc.tile_pool(name="pip", bufs=1, space="PSUM") as pip,
        ):
            # ---- load weights
            img_sb = cp.tile([128, 16, B], BF16)
            nc.sync.dma_start(img_sb[:], imgT[:])
            whh_sb = cp.tile([128, 4, 16, 128], BF16)
            nc.sync.dma_start(whh_sb[:], Whh[:])
            wih_sb = cp.tile([128, KX, 16, 128], BF16)
            nc.sync.dma_start(wih_sb[:], Wih[:])
            bg_sb = cp.tile([128, 16], F32)
            nc.sync.dma_start(bg_sb[:], bg[:])

            # ---- init MLPs -> hT0 (bf16) / cT0 (f32), shape [128, 4, B]
            cT = sp.tile([128, 4, B], F32)   # persistent cell state
            hT0 = sp.tile([128, 4, B], BF16)

            def init_mlp(W1d, b1d, W2d, b2d, out_ap, out_dtype):
                w1 = cp.tile([128, 16, 8, 128], BF16, tag="w1" + W1d.name)
                nc.sync.dma_start(w1[:], W1d[:])
                b1 = cp.tile([128, 8], F32, tag="b1" + b1d.name)
                nc.sync.dma_start(b1[:], b1d[:])
                w2 = cp.tile([128, 8, 4, 128], BF16, tag="w2" + W2d.name)
                nc.sync.dma_start(w2[:], W2d[:])
                b2 = cp.tile([128, 4], F32, tag="b2" + b2d.name)
                nc.sync.dma_start(b2[:], b2d[:])
                ps1 = pip.tile([128, 8, B], F32, tag="ps1")
                for mt in range(8):
                    for kt in range(16):
                        nc.tensor.matmul(ps1[:, mt, :], w1[:, kt, mt, :],
                                         img_sb[:, kt, :],
                                         start=(kt == 0), stop=(kt == 15))
                h1 = ewp.tile([128, 8, B], BF16, tag="h1mlp")
                for mt in range(8):
                    nc.scalar.activation(h1[:, mt, :], ps1[:, mt, :], AF.Relu,
                                         bias=b1[:, mt:mt + 1])
                ps2 = pip.tile([128, 4, B], F32, tag="ps2")
                for mt in range(4):
                    for kt in range(8):
                        nc.tensor.matmul(ps2[:, mt, :], w2[:, kt, mt, :],
                                         h1[:, kt, :],
                                         start=(kt == 0), stop=(kt == 7))
                for mt in range(4):
                    nc.scalar.activation(out_ap[:, mt, :], ps2[:, mt, :],
                                         AF.Relu, bias=b2[:, mt:mt + 1])

            init_mlp(Wh1, bh1, Wh2, bh2, hT0, BF16)
            init_mlp(Wc1, bc1, Wc2, bc2, cT, F32)

            # ---- interleaved per-call input projection machinery
            call_xg = {}

            def proj_closures(k):
                """Returns emission closures: x DMA + 16 m-tile projections."""
                L = LS[k]
                st = {}

                def start():
                    x_sb = xp.tile([128, KX, L * B], BF16, tag="x")
                    nc.sync.dma_start(
                        x_sb[:], xt[:, :, POS0[k] * B:(POS0[k] + L) * B])
                    xg_sb = xgp.tile([128, 16, L, B], F32, tag="xg")
                    st["x"] = x_sb
                    call_xg[k] = xg_sb

                def m_op(m):
                    x_sb = st["x"]
                    xg_sb = call_xg[k]
                    pp = ppp.tile([128, L * B], F32, tag="pp")
                    for kt in range(KX):
                        nc.tensor.matmul(pp[:], wih_sb[:, kt, m, :],
                                         x_sb[:, kt, :],
                                         start=(kt == 0), stop=(kt == KX - 1))
                    nc.scalar.activation(
                        xg_sb[:, m].rearrange("p l b -> p (l b)"), pp[:],
                        AF.Identity, bias=bg_sb[:, m:m + 1])

                return [start] + [
                    (lambda m=m: m_op(m)) for m in range(16)]

            from collections import deque
            pq = deque()
            for c in proj_closures(0):
                c()
            for c in proj_closures(1):
                c()

            # ---- the chain
            prev_h = None  # (tile, L) of previous call
            for k in range(len(CALLS)):
                L = LS[k]
                if k + 2 < len(CALLS):
                    pq.extend(proj_closures(k + 2))
                xg_sb = call_xg.pop(k)
                xgv = xg_sb.rearrange("p (g j) l b -> p g j l b", g=4)
                h_sb = hp.tile([128, 4, L, B], BF16, tag="h")
                # proj pop rate: drain queue over this call's steps
                rate = max(1, -(-len(pq) // max(1, 2 * L)))

                for s in range(L):
                    if s == 0:
                        if prev_h is None:
                            hsrc = lambda kt: hT0[:, kt, :]
                        else:
                            ph, pL = prev_h
                            hsrc = lambda kt, ph=ph, pL=pL: ph[:, kt, pL - 1, :]
                    else:
                        hsrc = lambda kt, s=s: h_sb[:, kt, s - 1, :]

                    # two psum tiles (separate banks): k-halves accumulate
                    # independently; groups within a bank stay consecutive
                    # (start=True clears has_written bank-wide).
                    pgA = pgp.tile([128, 4, 4, B], F32, tag="pgA")
                    pgB = pgp.tile([128, 4, 4, B], F32, tag="pgB")
                    for (jlo, jhi), kts in (((0, 2), (0, 2)), ((0, 2), (2, 4)),
                                            ((2, 4), (0, 2)), ((2, 4), (2, 4))):
                        pg_ = pgA if kts[0] == 0 else pgB
                        for g in range(4):
                            for jj in range(jlo, jhi):
                                for kt in range(*kts):
                                    nc.tensor.matmul(
                                        pg_[:, g, jj, :],
                                        whh_sb[:, kt, g * 4 + jj, :],
                                        hsrc(kt),
                                        start=(kt % 2 == 0), stop=(kt % 2 == 1),
                                        skip_group_check=True)
                    # elementwise, split in two j-halves
                    g_sb = ewp.tile([128, 4, 4, B], F32, tag="g")
                    s_sb = ewp.tile([128, 3, 4, B], F32, tag="s")
                    tg = ewp.tile([128, 4, B], F32, tag="tg")
                    tc_ = ewp.tile([128, 4, B], F32, tag="tc")
                    tmp = ewp.tile([128, 4, B], F32, tag="tmp")
                    for jh in (0, 1):
                        ch = slice(2 * jh, 2 * jh + 2)
                        nc.vector.tensor_tensor(
                            g_sb[:, :, ch, :], pgA[:, :, ch, :],
                            xgv[:, :, ch, s, :], ALU.add)
                        nc.vector.tensor_tensor(
                            g_sb[:, :, ch, :], pgB[:, :, ch, :],
                            g_sb[:, :, ch, :], ALU.add)
                        nc.scalar.activation(
                            s_sb[:, :, ch, :], g_sb[:, 0:3, ch, :], AF.Sigmoid)
                        nc.scalar.activation(
                            tg[:, ch, :], g_sb[:, 3, ch, :], AF.Tanh)
                        nc.vector.tensor_tensor(
                            tmp[:, ch, :], s_sb[:, 0, ch, :], tg[:, ch, :],
                            ALU.mult)
                        nc.vector.tensor_tensor(
                            cT[:, ch, :], s_sb[:, 1, ch, :], cT[:, ch, :],
                            ALU.mult)
                        nc.vector.tensor_tensor(
                            cT[:, ch, :], cT[:, ch, :], tmp[:, ch, :], ALU.add)
                        nc.scalar.activation(
                            tc_[:, ch, :], cT[:, ch, :], AF.Tanh)
                        nc.vector.tensor_tensor(
                            h_sb[:, ch, s, :], s_sb[:, 2, ch, :],
                            tc_[:, ch, :], ALU.mult)
                    for _ in range(2 * rate):
                        if pq:
                            pq.popleft()()
                nc.sync.dma_start(oh[:, :, POS0[k]:POS0[k] + L, :],
                                  h_sb[:, :, 0:L, :])
                prev_h = (h_sb, L)
            while pq:
                pq.popleft()()
    nc.compile()
    return nc


def build_fc():
    """FC head NEFF: logits[r, v] = y[r] @ Wfc[:, vshard] + bfc, per core."""
    nc = bacc.Bacc()
    yT = nc.dram_tensor("yT", [128, 8, RPAD], BF16, kind="ExternalInput")
    Wfc = nc.dram_tensor("Wfct", [128, 8, VL], BF16, kind="ExternalInput")
    bfc = nc.dram_tensor("bfcr", [128, VL], F32, kind="ExternalInput")
    out = nc.dram_tensor("logits", [RPAD, VL], F32, kind="ExternalOutput")
    with tile.TileContext(nc) as tc:
        with (
            tc.tile_pool(name="const", bufs=1) as cp,
            tc.tile_pool(name="ob", bufs=4) as op,
            tc.tile_pool(name="ps", bufs=4, space="PSUM") as pp,
        ):
            y_sb = cp.tile([128, 8, RPAD], BF16)
            nc.sync.dma_start(y_sb[:], yT[:])
            w_sb = cp.tile([128, 8, VL], BF16)
            nc.sync.dma_start(w_sb[:], Wfc[:])
            b_sb = cp.tile([128, VL], F32)
            nc.sync.dma_start(b_sb[:], bfc[:])
            chunks = [(c0, min(512, VL - c0)) for c0 in range(0, VL, 512)]
            for mt in range(RPAD // 128):
                for (c0, cs) in chunks:
                    ps = pp.tile([128, 512], F32, tag="ps")
                    for kt in range(8):
                        nc.tensor.matmul(
                            ps[:, :cs], y_sb[:, kt, mt * 128:(mt + 1) * 128],
                            w_sb[:, kt, c0:c0 + cs],
                            start=(kt == 0), stop=(kt == 7))
                    o_sb = op.tile([128, 512], F32, tag="o")
                    nc.vector.tensor_tensor(o_sb[:, :cs], ps[:, :cs],
                                            b_sb[:, c0:c0 + cs], ALU.add)
                    nc.sync.dma_start(
                        out[mt * 128:(mt + 1) * 128, c0:c0 + cs], o_sb[:, :cs])
    nc.compile()
    return nc


# ---------------------------------------------------------------- runner

_CACHE = {}


class _Runner:
    """Compile a Bacc module once into a sharded PJRT executable over the 8
    cores; allow warm re-execution for timing (device-resident inputs)."""

    def __init__(self, nc):
        import jax
        from jax.sharding import Mesh, PartitionSpec, NamedSharding
        from jax.experimental.shard_map import shard_map
        from concourse import bass2jax, mybir as _mb
        bass2jax.install_neuronx_cc_hook()
        self.jax = jax
        self.nc = nc
        partition_name = (nc.partition_id_tensor.name
                          if nc.partition_id_tensor else None)
        in_names, out_names, out_avals, zero_outs = [], [], [], []
        for alloc in nc.m.functions[0].allocations:
            if not isinstance(alloc, _mb.MemoryLocationSet):
                continue
            name = alloc.memorylocations[0].name
            if alloc.kind == "ExternalInput":
                if name != partition_name:
                    in_names.append(name)
            elif alloc.kind == "ExternalOutput":
                shape = tuple(alloc.tensor_shape)
                dtype = _mb.dt.np(alloc.dtype)
                out_names.append(name)
                out_avals.append(jax.core.ShapedArray(shape, dtype))
                zero_outs.append(np.zeros(shape, dtype))
        self.in_names = list(in_names)
        self.out_names = out_names
        self.out_avals = out_avals
        self.zero_outs = zero_outs
        n_params = len(in_names)
        all_in = in_names + out_names
        if partition_name is not None:
            all_in.append(partition_name)

        def _body(*args):
            operands = list(args)
            if partition_name is not None:
                operands.append(bass2jax.partition_id_tensor())
            return tuple(bass2jax._bass_exec_p.bind(
                *operands,
                out_avals=tuple(out_avals),
                in_names=tuple(all_in),
                out_names=tuple(out_names),
                lowering_input_output_aliases=(),
                sim_require_finite=True,
                sim_require_nnan=True,
                nc=nc,
            ))

        devices = jax.devices()[:NCORES]
        self.mesh = Mesh(np.asarray(devices), ("core",))
        self.sharding = NamedSharding(self.mesh, PartitionSpec("core"))
        n_in = n_params + len(out_names)
        self.sharded = jax.jit(shard_map(
            _body, mesh=self.mesh,
            in_specs=(PartitionSpec("core"),) * n_in,
            out_specs=(PartitionSpec("core"),) * len(out_names),
            check_rep=False), keep_unused=True)
        self._zeros_dev = None

    def stage(self, in_maps):
        """host->device transfer of per-core inputs; returns device args."""
        jax = self.jax
        concat = [np.concatenate([np.asarray(m[n]) for m in in_maps], axis=0)
                  for n in self.in_names]
        args = [jax.device_put(a, self.sharding) for a in concat]
        if self._zeros_dev is None:
            self._zeros_dev = [
                jax.device_put(
                    np.zeros((NCORES * z.shape[0], *z.shape[1:]), z.dtype),
                    self.sharding) for z in self.zero_outs]
        args += self._zeros_dev
        for a in args:
            a.block_until_ready()
        return args

    def execute(self, args):
        outs = self.sharded(*args)
        for o in outs:
            o.block_until_ready()
        return outs

    def run(self, in_maps, time_reps=0):
        import time as _t
        args = self.stage(in_maps)
        outs = self.execute(args)  # cold (compiles first time)
        if time_reps:
            best = float("inf")
            for _ in range(time_reps):
                t0 = _t.perf_counter()
                outs = self.execute(args)
                best = min(best, _t.perf_counter() - t0)
            _run.times.append(int(best * 1e9))
        res = []
        for c in range(NCORES):
            res.append({
                name: np.asarray(outs[i]).reshape(
                    NCORES, *self.out_avals[i].shape)[c]
                for i, name in enumerate(self.out_names)})
        return res


def _get_nc(key):
    if key not in _CACHE:
        nc = build_fc() if key == "fc" else build_chain(key)
        _CACHE[key] = _Runner(nc)
    return _CACHE[key]


def _run(runner, in_maps, trace=False):
    return runner.run(in_maps, time_reps=3 if trace else 0)


_run.times = []


def kernel(**inputs):
    trace = bool(int(os.environ.get("CAPNET_TRACE", "0")))
    _run.times = []
    inp = {k: np.asarray(v) for k, v in inputs.items()}
    per_dir = _chain_host_inputs(inp)

    # ---- phase 1: layer-0 chains (core 0 fwd, core 1 bwd)
    nc0 = _get_nc(2)
    maps0 = []
    for c in range(NCORES):
        d = c % 2
        m = {k: per_dir[d][k] for k in ("imgT", "Wh1t", "bh1t", "Wh2t", "bh2t",
                                        "Wc1t", "bc1t", "Wc2t", "bc2t")}
        m["Whh"] = per_dir[d]["Whh0"]
        m["Wih"] = per_dir[d]["Wih0"]
        m["bg"] = per_dir[d]["bg0"]
        m["xt"] = _x0_arranged(inp, rev=(d == 1)) if c < 2 else None
        maps0.append(m)
    maps0[0]["xt"] = _x0_arranged(inp, rev=False)
    maps0[1]["xt"] = _x0_arranged(inp, rev=True)
    for c in range(2, NCORES):
        maps0[c]["xt"] = maps0[c % 2]["xt"]
    res0 = _run(nc0, maps0, trace=trace)
    h0f = _oh_to_HposB(res0[0]["oh"])
    h0b = _oh_to_HposB(res0[1]["oh"])

    # ---- phase 2: layer-1 chains
    nc1 = _get_nc(8)
    maps1 = []
    for c in range(NCORES):
        d = c % 2
        m = {k: per_dir[d][k] for k in ("imgT", "Wh1t", "bh1t", "Wh2t", "bh2t",
                                        "Wc1t", "bc1t", "Wc2t", "bc2t")}
        m["Whh"] = per_dir[d]["Whh1"]
        m["Wih"] = per_dir[d]["Wih1"]
        m["bg"] = per_dir[d]["bg1"]
        maps1.append(m)
    x1f = _x1_arranged(h0f, h0b, rev=False)
    x1b = _x1_arranged(h0f, h0b, rev=True)
    for c in range(NCORES):
        maps1[c]["xt"] = x1f if c % 2 == 0 else x1b
    res1 = _run(nc1, maps1, trace=trace)
    h1f = _oh_to_HposB(res1[0]["oh"])
    h1b = _oh_to_HposB(res1[1]["oh"])

    # ---- phase 3: FC head (vocab-sharded)
    ncf = _get_nc("fc")
    yT = _y_assemble(h1f, h1b)
    Wfc = inp["Wfc"].astype(np.float32)
    bfc = inp["bfc"].astype(np.float32)
    mapsf = []
    for c in range(NCORES):
        v0 = c * VL
        wt = np.ascontiguousarray(
            Wfc[:, v0:v0 + VL].reshape(8, 128, VL).transpose(1, 0, 2)
        ).astype(nbf)
        bt = np.broadcast_to(bfc[v0:v0 + VL], (128, VL)).copy()
        mapsf.append({"yT": yT, "Wfct": wt, "bfcr": bt})
    resf = _run(ncf, mapsf, trace=trace)

    logits = np.empty((N, T, B, V), np.float32)
    for c in range(NCORES):
        logits[:, :, :, c * VL:(c + 1) * VL] = (
            resf[c]["logits"][:800].reshape(N, T, B, VL))
    return logits


# revision 19
# speedup vs baseline: 76.5730x; 76.5730x over previous
"""Trainium2 Bass kernel for nn_CaptionNet_23467701305971.

Model: image-captioning net. init MLPs -> 2-layer biLSTM with a redundant
prefix-recomputation state chain (50 sequential calls, 275 LSTM steps per
direction-chain) -> big FC head to vocab 30000.

Strategy (8 NeuronCores):
  - The 4 direction-chains (l0f, l0b, l1f, l1b) are strictly sequential
    inside, but l0f/l0b are independent and l1f/l1b depend on l0 outputs.
  - Phase 1: chain NEFF (one SPMD program, role differences are pure DATA):
    core 0 runs the layer-0 forward chain, core 1 the layer-0 backward chain
    (backward = same program on time-reversed per-call inputs).
  - Host glue: assemble layer-1 inputs x1 = concat(of, ob) in consumption
    order per direction.
  - Phase 2: same chain program (wider input dim) runs layer-1 fwd/bwd on
    cores 0/1.
  - Phase 3: FC head, vocab-sharded across all 8 cores.
  - All matmuls bf16 with fp32 PSUM accumulation; cell state c and gate
    pre-activations stay fp32.  Measured numeric error vs the fp32
    reference: ~4e-3 relative L2.

Kernel layout notes:
  - Everything is "transposed": H lives on SBUF partitions. The recurrent
    matmul is weight-stationary: 64 (LDW+MM) pairs of [128k x 128m] @ [128k,
    16batch] per step, gates land on partitions so sigmoid/tanh run on 128
    lanes.
  - Gate order is host-permuted to (i, f, o, g) so one ACT op covers all
    sigmoids.
  - The per-call input projection (xg = x @ Wih + b) is emitted interleaved
    with chain steps two calls ahead, filling PE bubbles left by the
    elementwise chain.
"""

import os
import sys
import numpy as np
import ml_dtypes

sys.path.insert(0, "/opt/trn_rl_repo")

import concourse.bass as bass  # noqa: E402
from concourse import bacc  # noqa: E402
import concourse.tile as tile  # noqa: E402
import concourse.mybir as mybir  # noqa: E402

BF16 = mybir.dt.bfloat16
F32 = mybir.dt.float32
AF = mybir.ActivationFunctionType
ALU = mybir.AluOpType

B, N, T, H, E, V, F = 16, 5, 10, 512, 250, 30000, 2048
CALLS = [(t, n) for t in range(T) for n in range(N)]
LS = [t + 1 for (t, n) in CALLS]
POS0 = np.concatenate([[0], np.cumsum(LS)]).astype(int)
NPOS = int(POS0[-1])  # 275
NCORES = 8
VL = V // NCORES  # 3750
RPAD = 896  # 800 output rows padded to 7*128

nbf = ml_dtypes.bfloat16


# ---------------------------------------------------------------- host prep

def _perm_gates(W):
    """reorder gate blocks (i,f,g,o) -> (i,f,o,g) along the last axis."""
    Hh = W.shape[-1] // 4
    return np.concatenate(
        [W[..., :Hh], W[..., Hh:2 * Hh], W[..., 3 * Hh:], W[..., 2 * Hh:3 * Hh]],
        axis=-1)


def _tile_w(W, KX, MT):
    """[Din, MT*128] -> [128, KX, MT, 128] bf16 stationary tiles."""
    Din, M = W.shape
    assert M == MT * 128
    Wp = np.zeros((KX * 128, M), np.float32)
    Wp[:Din] = W
    return np.ascontiguousarray(
        Wp.reshape(KX, 128, MT, 128).transpose(1, 0, 2, 3)).astype(nbf)


def _tile_b(b, MT):
    return np.ascontiguousarray(b.reshape(MT, 128).T).astype(np.float32)


def _chain_host_inputs(inp):
    """Per-core input dicts for the two chain phases (minus the x inputs)."""
    com = {
        "imgT": np.ascontiguousarray(
            inp["img"].T.reshape(16, 128, B).transpose(1, 0, 2)).astype(nbf),
        "Wh1t": _tile_w(inp["Wh1"], 16, 8), "bh1t": _tile_b(inp["bh1"], 8),
        "Wh2t": _tile_w(inp["Wh2"], 8, 4), "bh2t": _tile_b(inp["bh2"], 4),
        "Wc1t": _tile_w(inp["Wc1"], 16, 8), "bc1t": _tile_b(inp["bc1"], 8),
        "Wc2t": _tile_w(inp["Wc2"], 8, 4), "bc2t": _tile_b(inp["bc2"], 4),
    }
    per_dir = {}
    for d, sfx in ((0, "f"), (1, "b")):
        per_dir[d] = dict(com)
    for d, sfx in ((0, "f"), (1, "b")):
        per_dir[d]["Whh0"] = _tile_w(_perm_gates(inp["Whh0" + sfx]), 4, 16)
        per_dir[d]["Wih0"] = _tile_w(_perm_gates(inp["Wih0" + sfx]), 2, 16)
        per_dir[d]["bg0"] = _tile_b(_perm_gates(inp["b0" + sfx]), 16)
        per_dir[d]["Whh1"] = _tile_w(_perm_gates(inp["Whh1" + sfx]), 4, 16)
        per_dir[d]["Wih1"] = _tile_w(_perm_gates(inp["Wih1" + sfx]), 8, 16)
        per_dir[d]["bg1"] = _tile_b(_perm_gates(inp["b1" + sfx]), 16)
    return per_dir


def _x0_arranged(inp, rev):
    """layer-0 chain input, consumption order, transposed: [128, 2, NPOS*B]."""
    seq = inp["emb"][inp["caps"]].transpose(1, 2, 0, 3)  # [N, T, B, E]
    A = np.zeros((NPOS, B, 256), np.float32)
    for k, (t, n) in enumerate(CALLS):
        L = t + 1
        for s in range(L):
            tok = (L - 1 - s) if rev else s
            A[POS0[k] + s] = np.pad(seq[n, tok], ((0, 0), (0, 6)))
    return np.ascontiguousarray(
        A.reshape(NPOS * B, 2, 128).transpose(2, 1, 0)).astype(nbf)


def _oh_to_HposB(oh):
    """device oh [128, 4, NPOS, B] -> [H, NPOS, B] float32."""
    return oh.astype(np.float32).transpose(1, 0, 2, 3).reshape(H, NPOS, B)


def _x1_arranged(hf, hb, rev):
    """layer-1 chain input [128, 8, NPOS*B] bf16 from layer-0 outputs.

    hf/hb: [H, NPOS, B] layer-0 fwd/bwd chain outputs in their own
    consumption order (fwd slot s = natural s; bwd slot s = natural L-1-s).
    """
    pf = np.zeros(NPOS, int)
    pb = np.zeros(NPOS, int)
    for k, (t, n) in enumerate(CALLS):
        L = t + 1
        for s in range(L):  # s = consumption slot of the l1 chain
            nat = (L - 1 - s) if rev else s  # natural time of this slot
            pf[POS0[k] + s] = POS0[k] + nat          # fwd chain slot = nat
            pb[POS0[k] + s] = POS0[k] + (L - 1 - nat)  # bwd chain slot
    A = np.concatenate([hf[:, pf, :], hb[:, pb, :]], axis=0)  # [1024, NPOS, B]
    return np.ascontiguousarray(
        A.reshape(8, 128, NPOS * B).transpose(1, 0, 2)).astype(nbf)


def _y_assemble(h1f, h1b):
    """final FC input yT [128, 8, RPAD] bf16 from layer-1 chain outputs."""
    y = np.zeros((2 * H, RPAD), np.float32)
    for n in range(N):
        k = 45 + n
        L = 10
        for s in range(L):
            r = (n * T + s) * B
            y[:H, r:r + B] = h1f[:, POS0[k] + s, :]
            y[H:, r:r + B] = h1b[:, POS0[k] + L - 1 - s, :]
    return np.ascontiguousarray(
        y.reshape(8, 128, RPAD).transpose(1, 0, 2)).astype(nbf)


# ---------------------------------------------------------------- builders

def build_chain(KX):
    """Chain NEFF. KX = input k-tiles (2 for layer-0, 8 for layer-1).

    Inputs (per core): imgT, W/b for both init MLPs, xt [128,KX,NPOS*B] bf16
    (arranged consumption-order rows, transposed), Wih [128,KX,16,128],
    bg [128,16], Whh [128,4,16,128].
    Output: oh [128, 4, NPOS, B] bf16 (per-slot hidden states).
    """
    nc = bacc.Bacc()
    imgT = nc.dram_tensor("imgT", [128, 16, B], BF16, kind="ExternalInput")
    Wh1 = nc.dram_tensor("Wh1t", [128, 16, 8, 128], BF16, kind="ExternalInput")
    bh1 = nc.dram_tensor("bh1t", [128, 8], F32, kind="ExternalInput")
    Wh2 = nc.dram_tensor("Wh2t", [128, 8, 4, 128], BF16, kind="ExternalInput")
    bh2 = nc.dram_tensor("bh2t", [128, 4], F32, kind="ExternalInput")
    Wc1 = nc.dram_tensor("Wc1t", [128, 16, 8, 128], BF16, kind="ExternalInput")
    bc1 = nc.dram_tensor("bc1t", [128, 8], F32, kind="ExternalInput")
    Wc2 = nc.dram_tensor("Wc2t", [128, 8, 4, 128], BF16, kind="ExternalInput")
    bc2 = nc.dram_tensor("bc2t", [128, 4], F32, kind="ExternalInput")
    xt = nc.dram_tensor("xt", [128, KX, NPOS * B], BF16, kind="ExternalInput")
    Wih = nc.dram_tensor("Wih", [128, KX, 16, 128], BF16, kind="ExternalInput")
    bg = nc.dram_tensor("bg", [128, 16], F32, kind="ExternalInput")
    Whh = nc.dram_tensor("Whh", [128, 4, 16, 128], BF16, kind="ExternalInput")
    oh = nc.dram_tensor("oh", [128, 4, NPOS, B], BF16, kind="ExternalOutput")

    with tile.TileContext(nc) as tc:
        with (
            tc.tile_pool(name="const", bufs=1) as cp,
            tc.tile_pool(name="xp", bufs=3) as xp,
            tc.tile_pool(name="xgp", bufs=3) as xgp,
            tc.tile_pool(name="hp", bufs=2) as hp,
            tc.tile_pool(name="ewp", bufs=2) as ewp,
            tc.tile_pool(name="sp", bufs=1) as sp,
            tc.tile_pool(name="pgp", bufs=2, space="PSUM") as pgp,
            tc.tile_pool(name="ppp", bufs=2, space="PSUM") as ppp,
            tc.tile_pool(name="pip", bufs=1, space="PSUM") as pip,
        ):
            # ---- load weights
            img_sb = cp.tile([128, 16, B], BF16)
            nc.sync.dma_start(img_sb[:], imgT[:])
            whh_sb = cp.tile([128, 4, 16, 128], BF16)
            nc.sync.dma_start(whh_sb[:], Whh[:])
            wih_sb = cp.tile([128, KX, 16, 128], BF16)
            nc.sync.dma_start(wih_sb[:], Wih[:])
            bg_sb = cp.tile([128, 16], F32)
            nc.sync.dma_start(bg_sb[:], bg[:])

            # ---- init MLPs -> hT0 (bf16) / cT0 (f32), shape [128, 4, B]
            cT = sp.tile([128, 4, B], F32)   # persistent cell state
            hT0 = sp.tile([128, 4, B], BF16)

            def init_mlp(W1d, b1d, W2d, b2d, out_ap, out_dtype):
                w1 = cp.tile([128, 16, 8, 128], BF16, tag="w1" + W1d.name)
                nc.sync.dma_start(w1[:], W1d[:])
                b1 = cp.tile([128, 8], F32, tag="b1" + b1d.name)
                nc.sync.dma_start(b1[:], b1d[:])
                w2 = cp.tile([128, 8, 4, 128], BF16, tag="w2" + W2d.name)
                nc.sync.dma_start(w2[:], W2d[:])
                b2 = cp.tile([128, 4], F32, tag="b2" + b2d.name)
                nc.sync.dma_start(b2[:], b2d[:])
                ps1 = pip.tile([128, 8, B], F32, tag="ps1")
                for mt in range(8):
                    for kt in range(16):
                        nc.tensor.matmul(ps1[:, mt, :], w1[:, kt, mt, :],
                                         img_sb[:, kt, :],
                                         start=(kt == 0), stop=(kt == 15))
                h1 = ewp.tile([128, 8, B], BF16, tag="h1mlp")
                for mt in range(8):
                    nc.scalar.activation(h1[:, mt, :], ps1[:, mt, :], AF.Relu,
                                         bias=b1[:, mt:mt + 1])
                ps2 = pip.tile([128, 4, B], F32, tag="ps2")
                for mt in range(4):
                    for kt in range(8):
                        nc.tensor.matmul(ps2[:, mt, :], w2[:, kt, mt, :],
                                         h1[:, kt, :],
                                         start=(kt == 0), stop=(kt == 7))
                for mt in range(4):
                    nc.scalar.activation(out_ap[:, mt, :], ps2[:, mt, :],
                                         AF.Relu, bias=b2[:, mt:mt + 1])

            init_mlp(Wh1, bh1, Wh2, bh2, hT0, BF16)
            init_mlp(Wc1, bc1, Wc2, bc2, cT, F32)

            # ---- interleaved per-call input projection machinery
            call_xg = {}

            def proj_closures(k):
                """Returns emission closures: x DMA + 16 m-tile projections."""
                L = LS[k]
                st = {}

                def start():
                    x_sb = xp.tile([128, KX, L * B], BF16, tag="x")
                    nc.sync.dma_start(
                        x_sb[:], xt[:, :, POS0[k] * B:(POS0[k] + L) * B])
                    xg_sb = xgp.tile([128, 16, L, B], F32, tag="xg")
                    st["x"] = x_sb
                    call_xg[k] = xg_sb

                def m_op(m):
                    x_sb = st["x"]
                    xg_sb = call_xg[k]
                    pp = ppp.tile([128, L * B], F32, tag="pp")
                    for kt in range(KX):
                        nc.tensor.matmul(pp[:], wih_sb[:, kt, m, :],
                                         x_sb[:, kt, :],
                                         start=(kt == 0), stop=(kt == KX - 1))
                    nc.scalar.activation(
                        xg_sb[:, m].rearrange("p l b -> p (l b)"), pp[:],
                        AF.Identity, bias=bg_sb[:, m:m + 1])

                return [start] + [
                    (lambda m=m: m_op(m)) for m in range(16)]

            from collections import deque
            pq = deque()
            for c in proj_closures(0):
                c()
            for c in proj_closures(1):
                c()

            # ---- the chain
            prev_h = None  # (tile, L) of previous call
            for k in range(len(CALLS)):
                L = LS[k]
                if k + 2 < len(CALLS):
                    pq.extend(proj_closures(k + 2))
                xg_sb = call_xg.pop(k)
                xgv = xg_sb.rearrange("p (g j) l b -> p g j l b", g=4)
                h_sb = hp.tile([128, 4, L, B], BF16, tag="h")
                # proj pop rate: drain queue over this call's steps
                rate = max(1, -(-len(pq) // max(1, 2 * L)))

                for s in range(L):
                    if s == 0:
                        if prev_h is None:
                            hsrc = lambda kt: hT0[:, kt, :]
                        else:
                            ph, pL = prev_h
                            hsrc = lambda kt, ph=ph, pL=pL: ph[:, kt, pL - 1, :]
                    else:
                        hsrc = lambda kt, s=s: h_sb[:, kt, s - 1, :]

                    # two psum tiles (separate banks): k-halves accumulate
                    # independently; groups within a bank stay consecutive
                    # (start=True clears has_written bank-wide).
                    pgA = pgp.tile([128, 4, 4, B], F32, tag="pgA")
                    pgB = pgp.tile([128, 4, 4, B], F32, tag="pgB")
                    for (jlo, jhi), kts in (((0, 2), (0, 2)), ((0, 2), (2, 4)),
                                            ((2, 4), (0, 2)), ((2, 4), (2, 4))):
                        pg_ = pgA if kts[0] == 0 else pgB
                        for g in range(4):
                            for jj in range(jlo, jhi):
                                for kt in range(*kts):
                                    nc.tensor.matmul(
                                        pg_[:, g, jj, :],
                                        whh_sb[:, kt, g * 4 + jj, :],
                                        hsrc(kt),
                                        start=(kt % 2 == 0), stop=(kt % 2 == 1),
                                        skip_group_check=True)
                    # elementwise, split in two j-halves
                    g_sb = ewp.tile([128, 4, 4, B], F32, tag="g")
                    s_sb = ewp.tile([128, 3, 4, B], F32, tag="s")
                    tg = ewp.tile([128, 4, B], F32, tag="tg")
                    tc_ = ewp.tile([128, 4, B], F32, tag="tc")
                    tmp = ewp.tile([128, 4, B], F32, tag="tmp")
                    for jh in (0, 1):
                        ch = slice(2 * jh, 2 * jh + 2)
                        nc.vector.tensor_tensor(
                            g_sb[:, :, ch, :], pgA[:, :, ch, :],
                            xgv[:, :, ch, s, :], ALU.add)
                        nc.vector.tensor_tensor(
                            g_sb[:, :, ch, :], pgB[:, :, ch, :],
                            g_sb[:, :, ch, :], ALU.add)
                        nc.scalar.activation(
                            s_sb[:, :, ch, :], g_sb[:, 0:3, ch, :], AF.Sigmoid)
                        nc.scalar.activation(
                            tg[:, ch, :], g_sb[:, 3, ch, :], AF.Tanh)
                        nc.vector.tensor_tensor(
                            tmp[:, ch, :], s_sb[:, 0, ch, :], tg[:, ch, :],
                            ALU.mult)
                        nc.vector.tensor_tensor(
                            cT[:, ch, :], s_sb[:, 1, ch, :], cT[:, ch, :],
                            ALU.mult)
                        nc.vector.tensor_tensor(
                            cT[:, ch, :], cT[:, ch, :], tmp[:, ch, :], ALU.add)
                        nc.scalar.activation(
                            tc_[:, ch, :], cT[:, ch, :], AF.Tanh)
                        nc.vector.tensor_tensor(
                            h_sb[:, ch, s, :], s_sb[:, 2, ch, :],
                            tc_[:, ch, :], ALU.mult)
                    for _ in range(2 * rate):
                        if pq:
                            pq.popleft()()
                nc.sync.dma_start(oh[:, :, POS0[k]:POS0[k] + L, :],
                                  h_sb[:, :, 0:L, :])
                prev_h = (h_sb, L)
            while pq:
                pq.popleft()()
    nc.compile()
    return nc


def build_fc():
    """FC head NEFF: logits[r, v] = y[r] @ Wfc[:, vshard] + bfc, per core."""
    nc = bacc.Bacc()
    yT = nc.dram_tensor("yT", [128, 8, RPAD], BF16, kind="ExternalInput")
    Wfc = nc.dram_tensor("Wfct", [128, 8, VL], BF16, kind="ExternalInput")
    bfc = nc.dram_tensor("bfcr", [128, VL], F32, kind="ExternalInput")
    out = nc.dram_tensor("logits", [RPAD, VL], F32, kind="ExternalOutput")
    with tile.TileContext(nc) as tc:
        with (
            tc.tile_pool(name="const", bufs=1) as cp,
            tc.tile_pool(name="ob", bufs=4) as op,
            tc.tile_pool(name="ps", bufs=4, space="PSUM") as pp,
        ):
            y_sb = cp.tile([128, 8, RPAD], BF16)
            nc.sync.dma_start(y_sb[:], yT[:])
            w_sb = cp.tile([128, 8, VL], BF16)
            nc.sync.dma_start(w_sb[:], Wfc[:])
            b_sb = cp.tile([128, VL], F32)
            nc.sync.dma_start(b_sb[:], bfc[:])
            chunks = [(c0, min(512, VL - c0)) for c0 in range(0, VL, 512)]
            for mt in range(RPAD // 128):
                for (c0, cs) in chunks:
                    ps = pp.tile([128, 512], F32, tag="ps")
                    for kt in range(8):
                        nc.tensor.matmul(
                            ps[:, :cs], y_sb[:, kt, mt * 128:(mt + 1) * 128],
                            w_sb[:, kt, c0:c0 + cs],
                            start=(kt == 0), stop=(kt == 7))
                    o_sb = op.tile([128, 512], F32, tag="o")
                    nc.vector.tensor_tensor(o_sb[:, :cs], ps[:, :cs],
                                            b_sb[:, c0:c0 + cs], ALU.add)
                    nc.sync.dma_start(
                        out[mt * 128:(mt + 1) * 128, c0:c0 + cs], o_sb[:, :cs])
    nc.compile()
    return nc


# ---------------------------------------------------------------- runner

_CACHE = {}


class _Runner:
    """Compile a Bacc module once into a sharded PJRT executable over the 8
    cores; allow warm re-execution for timing (device-resident inputs)."""

    def __init__(self, nc):
        import jax
        from jax.sharding import Mesh, PartitionSpec, NamedSharding
        from jax.experimental.shard_map import shard_map
        from concourse import bass2jax, mybir as _mb
        bass2jax.install_neuronx_cc_hook()
        self.jax = jax
        self.nc = nc
        partition_name = (nc.partition_id_tensor.name
                          if nc.partition_id_tensor else None)
        in_names, out_names, out_avals, zero_outs = [], [], [], []
        self.in_specs = {}
        for alloc in nc.m.functions[0].allocations:
            if not isinstance(alloc, _mb.MemoryLocationSet):
                continue
            name = alloc.memorylocations[0].name
            if alloc.kind == "ExternalInput":
                if name != partition_name:
                    in_names.append(name)
                    self.in_specs[name] = (tuple(alloc.tensor_shape),
                                           _mb.dt.np(alloc.dtype))
            elif alloc.kind == "ExternalOutput":
                shape = tuple(alloc.tensor_shape)
                dtype = _mb.dt.np(alloc.dtype)
                out_names.append(name)
                out_avals.append(jax.core.ShapedArray(shape, dtype))
                zero_outs.append(np.zeros(shape, dtype))
        self.in_names = list(in_names)
        self.out_names = out_names
        self.out_avals = out_avals
        self.zero_outs = zero_outs
        n_params = len(in_names)
        all_in = in_names + out_names
        if partition_name is not None:
            all_in.append(partition_name)

        def _body(*args):
            operands = list(args)
            if partition_name is not None:
                operands.append(bass2jax.partition_id_tensor())
            return tuple(bass2jax._bass_exec_p.bind(
                *operands,
                out_avals=tuple(out_avals),
                in_names=tuple(all_in),
                out_names=tuple(out_names),
                lowering_input_output_aliases=(),
                sim_require_finite=True,
                sim_require_nnan=True,
                nc=nc,
            ))

        devices = jax.devices()[:NCORES]
        self.mesh = Mesh(np.asarray(devices), ("core",))
        self.sharding = NamedSharding(self.mesh, PartitionSpec("core"))
        n_in = n_params + len(out_names)
        self.sharded = jax.jit(shard_map(
            _body, mesh=self.mesh,
            in_specs=(PartitionSpec("core"),) * n_in,
            out_specs=(PartitionSpec("core"),) * len(out_names),
            check_rep=False), keep_unused=True)
        self._zeros_dev = None

    def warm(self):
        """trigger jit trace + neuronx compile with zero inputs."""
        zmap = {n: np.zeros(s, d) for n, (s, d) in self.in_specs.items()}
        self.run([zmap] * NCORES)

    def stage(self, in_maps):
        """host->device transfer of per-core inputs; returns device args."""
        jax = self.jax
        concat = [np.concatenate([np.asarray(m[n]) for m in in_maps], axis=0)
                  for n in self.in_names]
        args = [jax.device_put(a, self.sharding) for a in concat]
        if self._zeros_dev is None:
            self._zeros_dev = [
                jax.device_put(
                    np.zeros((NCORES * z.shape[0], *z.shape[1:]), z.dtype),
                    self.sharding) for z in self.zero_outs]
        args += self._zeros_dev
        for a in args:
            a.block_until_ready()
        return args

    def execute(self, args):
        outs = self.sharded(*args)
        for o in outs:
            o.block_until_ready()
        return outs

    def burst(self, args, reps=16, tries=3):
        """min total seconds for `reps` pipelined dispatches (async submit,
        block once at the end) — marginal per-exec isolates device time from
        the fixed dispatch floor."""
        import time as _t
        self.execute(args)  # warm
        best = float("inf")
        for _ in range(tries):
            t0 = _t.perf_counter()
            outs = None
            for _ in range(reps):
                outs = self.sharded(*args)
            for o in outs:
                o.block_until_ready()
            best = min(best, _t.perf_counter() - t0)
        return best / reps

    def run(self, in_maps, time_reps=0):
        args = self.stage(in_maps)
        outs = self.execute(args)  # cold (compiles first time)
        if time_reps:
            _run.times.append(int(self.burst(args) * 1e9))
        res = []
        for c in range(NCORES):
            res.append({
                name: np.asarray(outs[i]).reshape(
                    NCORES, *self.out_avals[i].shape)[c]
                for i, name in enumerate(self.out_names)})
        return res


import threading as _threading
_CACHE_LOCKS = {k: _threading.Lock() for k in (2, 8, "fc", "fused")}


def _get_nc(key):
    with _CACHE_LOCKS[key]:
        if key not in _CACHE:
            nc = build_fc() if key == "fc" else build_chain(key)
            _CACHE[key] = _Runner(nc)
    return _CACHE[key]


def _perm_tables():
    """static index tables for on-device x1 arrangement and y assembly."""
    pf_f = np.zeros(NPOS, np.int32); pb_f = np.zeros(NPOS, np.int32)
    pf_b = np.zeros(NPOS, np.int32); pb_b = np.zeros(NPOS, np.int32)
    for k, (t, n) in enumerate(CALLS):
        L = t + 1
        for s in range(L):
            pf_f[POS0[k] + s] = POS0[k] + s
            pb_f[POS0[k] + s] = POS0[k] + (L - 1 - s)
            nat = L - 1 - s
            pf_b[POS0[k] + s] = POS0[k] + nat
            pb_b[POS0[k] + s] = POS0[k] + (L - 1 - nat)
    yf = np.zeros(800 // B, np.int32); yb = np.zeros(800 // B, np.int32)
    for n in range(N):
        for s in range(T):
            yf[n * T + s] = POS0[45 + n] + s
            yb[n * T + s] = POS0[45 + n] + (T - 1 - s)
    return pf_f, pb_f, pf_b, pb_b, yf, yb


class _FusedRunner:
    """All three phases in ONE jitted dispatch: bass custom calls with XLA
    gather/concat glue and lax.all_gather for the cross-core handoffs."""

    IN0 = ["imgT", "Wh1t", "bh1t", "Wh2t", "bh2t", "Wc1t", "bc1t", "Wc2t",
           "bc2t", "xt", "Wih", "bg", "Whh"]

    def __init__(self):
        import jax
        import jax.numpy as jnp
        from jax import lax
        from jax.sharding import Mesh, PartitionSpec, NamedSharding
        from jax.experimental.shard_map import shard_map
        from concourse import bass2jax
        bass2jax.install_neuronx_cc_hook()
        self.jax = jax
        nc0 = build_chain(2)
        nc1 = build_chain(8)
        ncf = build_fc()
        pf_f, pb_f, pf_b, pb_b, yf, yb = _perm_tables()

        def bind(nc, in_map, out_shapes):
            names = list(in_map.keys())
            out_names = list(out_shapes.keys())
            operands = list(in_map.values())
            zero_outs = [jnp.zeros(s, d) for s, d in out_shapes.values()]
            operands += zero_outs
            all_names = names + out_names
            pn = nc.partition_id_tensor.name if nc.partition_id_tensor else None
            if pn is not None:
                operands.append(bass2jax.partition_id_tensor())
                all_names.append(pn)
            avals = tuple(jax.core.ShapedArray(s, d)
                          for s, d in out_shapes.values())
            outs = bass2jax._bass_exec_p.bind(
                *operands, out_avals=avals, in_names=tuple(all_names),
                out_names=tuple(out_names),
                lowering_input_output_aliases=(),
                sim_require_finite=True, sim_require_nnan=True, nc=nc)
            return dict(zip(out_names, outs))

        OH = ((128, 4, NPOS, B), np.dtype(ml_dtypes.bfloat16))
        LG = ((RPAD, VL), np.dtype(np.float32))

        def body(*args):
            a = dict(zip(self.in_names, args))
            com = {k: a[k] for k in self.IN0[:9]}
            o0 = bind(nc0, {**com, "xt": a["xt"], "Wih": a["Wih0"],
                            "bg": a["bg0"], "Whh": a["Whh0"]}, {"oh": OH})
            g = lax.all_gather(o0["oh"], "core")  # [8,128,4,NPOS,B]
            hf, hb = g[0], g[1]
            x1f = jnp.concatenate(
                [hf[:, :, pf_f, :], hb[:, :, pb_f, :]], axis=1)
            x1b = jnp.concatenate(
                [hf[:, :, pf_b, :], hb[:, :, pb_b, :]], axis=1)
            par = lax.axis_index("core") % 2
            xt1 = jnp.where(par == 0, x1f, x1b).reshape(128, 8, NPOS * B)
            o1 = bind(nc1, {**com, "xt": xt1, "Wih": a["Wih1"],
                            "bg": a["bg1"], "Whh": a["Whh1"]}, {"oh": OH})
            g1 = lax.all_gather(o1["oh"], "core")
            h1f, h1b = g1[0], g1[1]
            y = jnp.concatenate(
                [h1f[:, :, yf, :], h1b[:, :, yb, :]], axis=1)  # [128,8,50,B]
            y = y.reshape(128, 8, 800)
            yT = jnp.pad(y, ((0, 0), (0, 0), (0, RPAD - 800)))
            of = bind(ncf, {"yT": yT, "Wfct": a["Wfct"], "bfcr": a["bfcr"]},
                      {"logits": LG})
            return of["logits"]

        self.in_names = (self.IN0[:9] + ["xt", "Wih0", "bg0", "Whh0",
                                         "Wih1", "bg1", "Whh1",
                                         "Wfct", "bfcr"])
        devices = jax.devices()[:NCORES]
        self.mesh = Mesh(np.asarray(devices), ("core",))
        self.sharding = NamedSharding(self.mesh, PartitionSpec("core"))
        self.sharded = jax.jit(shard_map(
            body, mesh=self.mesh,
            in_specs=(PartitionSpec("core"),) * len(self.in_names),
            out_specs=PartitionSpec("core"), check_rep=False),
            keep_unused=True)

    def stage(self, in_maps):
        jax = self.jax
        concat = [np.concatenate([np.asarray(m[n]) for m in in_maps], axis=0)
                  for n in self.in_names]
        args = [jax.device_put(a, self.sharding) for a in concat]
        for a in args:
            a.block_until_ready()
        return args

    def execute(self, args):
        out = self.sharded(*args)
        out.block_until_ready()
        return out

    def run(self, in_maps, time_reps=0):
        import time as _t
        args = self.stage(in_maps)
        out = self.execute(args)
        if time_reps:
            best = float("inf")
            for _ in range(time_reps):
                t0 = _t.perf_counter()
                out = self.execute(args)
                best = min(best, _t.perf_counter() - t0)
            _run.times.append(int(best * 1e9))
        return np.asarray(out).reshape(NCORES, RPAD, VL)


def _run(runner, in_maps, trace=False):
    return runner.run(in_maps, time_reps=3 if trace else 0)


_run.times = []


def _fc_shards(inp):
    Wfc = inp["Wfc"].astype(np.float32)
    bfc = inp["bfc"].astype(np.float32)
    shards = []
    for c in range(NCORES):
        v0 = c * VL
        wt = np.ascontiguousarray(
            Wfc[:, v0:v0 + VL].reshape(8, 128, VL).transpose(1, 0, 2)
        ).astype(nbf)
        bt = np.broadcast_to(bfc[v0:v0 + VL], (128, VL)).copy()
        shards.append((wt, bt))
    return shards


def kernel(**inputs):
    trace = bool(int(os.environ.get("CAPNET_TRACE", "0")))
    _run.times = []
    inp = {k: np.asarray(v) for k, v in inputs.items()}
    if int(os.environ.get("CAPNET_FUSE", "0")):
        return _kernel_fused(inp, trace)
    return _kernel_3phase(inp, trace)


def _kernel_fused(inp, trace):
    per_dir = _chain_host_inputs(inp)
    if "fused" not in _CACHE:
        _CACHE["fused"] = _FusedRunner()
    fr = _CACHE["fused"]
    x0 = {0: _x0_arranged(inp, False), 1: _x0_arranged(inp, True)}
    fcs = _fc_shards(inp)
    maps = []
    for c in range(NCORES):
        d = c % 2
        m = {k: per_dir[d][k] for k in _FusedRunner.IN0[:9]}
        m["xt"] = x0[d]
        for tag in ("Wih", "bg", "Whh"):
            m[tag + "0"] = per_dir[d][tag + "0"]
            m[tag + "1"] = per_dir[d][tag + "1"]
        m["Wfct"], m["bfcr"] = fcs[c]
        maps.append(m)
    res = fr.run(maps, time_reps=3 if trace else 0)  # [8, RPAD, VL]
    logits = np.empty((N, T, B, V), np.float32)
    for c in range(NCORES):
        logits[:, :, :, c * VL:(c + 1) * VL] = (
            res[c][:800].reshape(N, T, B, VL))
    return logits


def _kernel_3phase(inp, trace):
    per_dir = _chain_host_inputs(inp)

    # ---- phase 1: layer-0 chains (core 0 fwd, core 1 bwd)
    nc0 = _get_nc(2)
    maps0 = []
    for c in range(NCORES):
        d = c % 2
        m = {k: per_dir[d][k] for k in ("imgT", "Wh1t", "bh1t", "Wh2t", "bh2t",
                                        "Wc1t", "bc1t", "Wc2t", "bc2t")}
        m["Whh"] = per_dir[d]["Whh0"]
        m["Wih"] = per_dir[d]["Wih0"]
        m["bg"] = per_dir[d]["bg0"]
        maps0.append(m)
    x0f = _x0_arranged(inp, rev=False)
    x0b = _x0_arranged(inp, rev=True)
    for c in range(NCORES):
        maps0[c]["xt"] = x0f if c % 2 == 0 else x0b
    res0 = _run(nc0, maps0, trace=trace)
    h0f = _oh_to_HposB(res0[0]["oh"])
    h0b = _oh_to_HposB(res0[1]["oh"])

    # ---- phase 2: layer-1 chains
    nc1 = _get_nc(8)
    maps1 = []
    for c in range(NCORES):
        d = c % 2
        m = {k: per_dir[d][k] for k in ("imgT", "Wh1t", "bh1t", "Wh2t", "bh2t",
                                        "Wc1t", "bc1t", "Wc2t", "bc2t")}
        m["Whh"] = per_dir[d]["Whh1"]
        m["Wih"] = per_dir[d]["Wih1"]
        m["bg"] = per_dir[d]["bg1"]
        maps1.append(m)
    x1f = _x1_arranged(h0f, h0b, rev=False)
    x1b = _x1_arranged(h0f, h0b, rev=True)
    for c in range(NCORES):
        maps1[c]["xt"] = x1f if c % 2 == 0 else x1b
    res1 = _run(nc1, maps1, trace=trace)
    h1f = _oh_to_HposB(res1[0]["oh"])
    h1b = _oh_to_HposB(res1[1]["oh"])

    # ---- phase 3: FC head (vocab-sharded)
    ncf = _get_nc("fc")
    yT = _y_assemble(h1f, h1b)
    fcs = _fc_shards(inp)
    mapsf = [{"yT": yT, "Wfct": fcs[c][0], "bfcr": fcs[c][1]}
             for c in range(NCORES)]
    resf = _run(ncf, mapsf, trace=trace)

    logits = np.empty((N, T, B, V), np.float32)
    for c in range(NCORES):
        logits[:, :, :, c * VL:(c + 1) * VL] = (
            resf[c]["logits"][:800].reshape(N, T, B, VL))
    return logits


# revision 21
# speedup vs baseline: 78.9325x; 1.0308x over previous
"""Trainium2 Bass kernel for nn_CaptionNet_23467701305971.

Model: image-captioning net. init MLPs -> 2-layer biLSTM with a redundant
prefix-recomputation state chain (50 sequential calls, 275 LSTM steps per
direction-chain) -> big FC head to vocab 30000.

Strategy (8 NeuronCores):
  - The 4 direction-chains (l0f, l0b, l1f, l1b) are strictly sequential
    inside, but l0f/l0b are independent and l1f/l1b depend on l0 outputs.
  - Phase 1: chain NEFF (one SPMD program, role differences are pure DATA):
    core 0 runs the layer-0 forward chain, core 1 the layer-0 backward chain
    (backward = same program on time-reversed per-call inputs).
  - Host glue: assemble layer-1 inputs x1 = concat(of, ob) in consumption
    order per direction.
  - Phase 2: same chain program (wider input dim) runs layer-1 fwd/bwd on
    cores 0/1.
  - Phase 3: FC head, vocab-sharded across all 8 cores.
  - All matmuls bf16 with fp32 PSUM accumulation; cell state c and gate
    pre-activations stay fp32.  Measured numeric error vs the fp32
    reference: ~4e-3 relative L2.

Kernel layout notes:
  - Everything is "transposed": H lives on SBUF partitions. The recurrent
    matmul is weight-stationary: 64 (LDW+MM) pairs of [128k x 128m] @ [128k,
    16batch] per step, gates land on partitions so sigmoid/tanh run on 128
    lanes.
  - Gate order is host-permuted to (i, f, o, g) so one ACT op covers all
    sigmoids.
  - The per-call input projection (xg = x @ Wih + b) is emitted interleaved
    with chain steps two calls ahead, filling PE bubbles left by the
    elementwise chain.
"""

import os
import sys
import numpy as np
import ml_dtypes

sys.path.insert(0, "/opt/trn_rl_repo")

import concourse.bass as bass  # noqa: E402
from concourse import bacc  # noqa: E402
import concourse.tile as tile  # noqa: E402
import concourse.mybir as mybir  # noqa: E402

BF16 = mybir.dt.bfloat16
F32 = mybir.dt.float32
AF = mybir.ActivationFunctionType
ALU = mybir.AluOpType

B, N, T, H, E, V, F = 16, 5, 10, 512, 250, 30000, 2048
CALLS = [(t, n) for t in range(T) for n in range(N)]
LS = [t + 1 for (t, n) in CALLS]
POS0 = np.concatenate([[0], np.cumsum(LS)]).astype(int)
NPOS = int(POS0[-1])  # 275
NCORES = 8
VL = V // NCORES  # 3750
RPAD = 896  # 800 output rows padded to 7*128

nbf = ml_dtypes.bfloat16


# ---------------------------------------------------------------- host prep

def _perm_gates(W):
    """reorder gate blocks (i,f,g,o) -> (i,f,o,g) along the last axis."""
    Hh = W.shape[-1] // 4
    return np.concatenate(
        [W[..., :Hh], W[..., Hh:2 * Hh], W[..., 3 * Hh:], W[..., 2 * Hh:3 * Hh]],
        axis=-1)


def _tile_w(W, KX, MT):
    """[Din, MT*128] -> [128, KX, MT, 128] bf16 stationary tiles."""
    Din, M = W.shape
    assert M == MT * 128
    Wp = np.zeros((KX * 128, M), np.float32)
    Wp[:Din] = W
    return np.ascontiguousarray(
        Wp.reshape(KX, 128, MT, 128).transpose(1, 0, 2, 3)).astype(nbf)


def _tile_b(b, MT):
    return np.ascontiguousarray(b.reshape(MT, 128).T).astype(np.float32)


def _chain_host_inputs(inp):
    """Per-core input dicts for the two chain phases (minus the x inputs)."""
    com = {
        "imgT": np.ascontiguousarray(
            inp["img"].T.reshape(16, 128, B).transpose(1, 0, 2)).astype(nbf),
        "Wh1t": _tile_w(inp["Wh1"], 16, 8), "bh1t": _tile_b(inp["bh1"], 8),
        "Wh2t": _tile_w(inp["Wh2"], 8, 4), "bh2t": _tile_b(inp["bh2"], 4),
        "Wc1t": _tile_w(inp["Wc1"], 16, 8), "bc1t": _tile_b(inp["bc1"], 8),
        "Wc2t": _tile_w(inp["Wc2"], 8, 4), "bc2t": _tile_b(inp["bc2"], 4),
    }
    per_dir = {}
    for d, sfx in ((0, "f"), (1, "b")):
        per_dir[d] = dict(com)
    for d, sfx in ((0, "f"), (1, "b")):
        per_dir[d]["Whh0"] = _tile_w(_perm_gates(inp["Whh0" + sfx]), 4, 16)
        per_dir[d]["Wih0"] = _tile_w(_perm_gates(inp["Wih0" + sfx]), 2, 16)
        per_dir[d]["bg0"] = _tile_b(_perm_gates(inp["b0" + sfx]), 16)
        per_dir[d]["Whh1"] = _tile_w(_perm_gates(inp["Whh1" + sfx]), 4, 16)
        per_dir[d]["Wih1"] = _tile_w(_perm_gates(inp["Wih1" + sfx]), 8, 16)
        per_dir[d]["bg1"] = _tile_b(_perm_gates(inp["b1" + sfx]), 16)
    return per_dir


def _x0_arranged(inp, rev):
    """layer-0 chain input, consumption order, transposed: [128, 2, NPOS*B]."""
    seq = inp["emb"][inp["caps"]].transpose(1, 2, 0, 3)  # [N, T, B, E]
    A = np.zeros((NPOS, B, 256), np.float32)
    for k, (t, n) in enumerate(CALLS):
        L = t + 1
        for s in range(L):
            tok = (L - 1 - s) if rev else s
            A[POS0[k] + s] = np.pad(seq[n, tok], ((0, 0), (0, 6)))
    return np.ascontiguousarray(
        A.reshape(NPOS * B, 2, 128).transpose(2, 1, 0)).astype(nbf)


def _oh_to_HposB(oh):
    """device oh [128, 4, NPOS, B] -> [H, NPOS, B] float32."""
    return oh.astype(np.float32).transpose(1, 0, 2, 3).reshape(H, NPOS, B)


def _x1_arranged(hf, hb, rev):
    """layer-1 chain input [128, 8, NPOS*B] bf16 from layer-0 outputs.

    hf/hb: [H, NPOS, B] layer-0 fwd/bwd chain outputs in their own
    consumption order (fwd slot s = natural s; bwd slot s = natural L-1-s).
    """
    pf = np.zeros(NPOS, int)
    pb = np.zeros(NPOS, int)
    for k, (t, n) in enumerate(CALLS):
        L = t + 1
        for s in range(L):  # s = consumption slot of the l1 chain
            nat = (L - 1 - s) if rev else s  # natural time of this slot
            pf[POS0[k] + s] = POS0[k] + nat          # fwd chain slot = nat
            pb[POS0[k] + s] = POS0[k] + (L - 1 - nat)  # bwd chain slot
    A = np.concatenate([hf[:, pf, :], hb[:, pb, :]], axis=0)  # [1024, NPOS, B]
    return np.ascontiguousarray(
        A.reshape(8, 128, NPOS * B).transpose(1, 0, 2)).astype(nbf)


def _y_assemble(h1f, h1b):
    """final FC input yT [128, 8, RPAD] bf16 from layer-1 chain outputs."""
    y = np.zeros((2 * H, RPAD), np.float32)
    for n in range(N):
        k = 45 + n
        L = 10
        for s in range(L):
            r = (n * T + s) * B
            y[:H, r:r + B] = h1f[:, POS0[k] + s, :]
            y[H:, r:r + B] = h1b[:, POS0[k] + L - 1 - s, :]
    return np.ascontiguousarray(
        y.reshape(8, 128, RPAD).transpose(1, 0, 2)).astype(nbf)


# ---------------------------------------------------------------- builders

def build_chain(KX):
    """Chain NEFF. KX = input k-tiles (2 for layer-0, 8 for layer-1).

    Inputs (per core): imgT, W/b for both init MLPs, xt [128,KX,NPOS*B] bf16
    (arranged consumption-order rows, transposed), Wih [128,KX,16,128],
    bg [128,16], Whh [128,4,16,128].
    Output: oh [128, 4, NPOS, B] bf16 (per-slot hidden states).
    """
    nc = bacc.Bacc()
    imgT = nc.dram_tensor("imgT", [128, 16, B], BF16, kind="ExternalInput")
    Wh1 = nc.dram_tensor("Wh1t", [128, 16, 8, 128], BF16, kind="ExternalInput")
    bh1 = nc.dram_tensor("bh1t", [128, 8], F32, kind="ExternalInput")
    Wh2 = nc.dram_tensor("Wh2t", [128, 8, 4, 128], BF16, kind="ExternalInput")
    bh2 = nc.dram_tensor("bh2t", [128, 4], F32, kind="ExternalInput")
    Wc1 = nc.dram_tensor("Wc1t", [128, 16, 8, 128], BF16, kind="ExternalInput")
    bc1 = nc.dram_tensor("bc1t", [128, 8], F32, kind="ExternalInput")
    Wc2 = nc.dram_tensor("Wc2t", [128, 8, 4, 128], BF16, kind="ExternalInput")
    bc2 = nc.dram_tensor("bc2t", [128, 4], F32, kind="ExternalInput")
    xt = nc.dram_tensor("xt", [128, KX, NPOS * B], BF16, kind="ExternalInput")
    Wih = nc.dram_tensor("Wih", [128, KX, 16, 128], BF16, kind="ExternalInput")
    bg = nc.dram_tensor("bg", [128, 16], F32, kind="ExternalInput")
    Whh = nc.dram_tensor("Whh", [128, 4, 16, 128], BF16, kind="ExternalInput")
    oh = nc.dram_tensor("oh", [128, 4, NPOS, B], BF16, kind="ExternalOutput")

    with tile.TileContext(nc) as tc:
        with (
            tc.tile_pool(name="const", bufs=1) as cp,
            tc.tile_pool(name="xp", bufs=3) as xp,
            tc.tile_pool(name="xgp", bufs=3) as xgp,
            tc.tile_pool(name="hp", bufs=2) as hp,
            tc.tile_pool(name="ewp", bufs=2) as ewp,
            tc.tile_pool(name="sp", bufs=1) as sp,
            tc.tile_pool(name="pgp", bufs=2, space="PSUM") as pgp,
            tc.tile_pool(name="ppp", bufs=2, space="PSUM") as ppp,
            tc.tile_pool(name="pip", bufs=1, space="PSUM") as pip,
        ):
            # ---- load weights
            img_sb = cp.tile([128, 16, B], BF16)
            nc.sync.dma_start(img_sb[:], imgT[:])
            whh_sb = cp.tile([128, 4, 16, 128], BF16)
            nc.sync.dma_start(whh_sb[:], Whh[:])
            wih_sb = cp.tile([128, KX, 16, 128], BF16)
            nc.sync.dma_start(wih_sb[:], Wih[:])
            bg_sb = cp.tile([128, 16], F32)
            nc.sync.dma_start(bg_sb[:], bg[:])

            # ---- init MLPs -> hT0 (bf16) / cT0 (f32), shape [128, 4, B]
            cT = sp.tile([128, 4, B], F32)   # persistent cell state
            hT0 = sp.tile([128, 4, B], BF16)

            def init_mlp(W1d, b1d, W2d, b2d, out_ap, out_dtype):
                w1 = cp.tile([128, 16, 8, 128], BF16, tag="w1" + W1d.name)
                nc.sync.dma_start(w1[:], W1d[:])
                b1 = cp.tile([128, 8], F32, tag="b1" + b1d.name)
                nc.sync.dma_start(b1[:], b1d[:])
                w2 = cp.tile([128, 8, 4, 128], BF16, tag="w2" + W2d.name)
                nc.sync.dma_start(w2[:], W2d[:])
                b2 = cp.tile([128, 4], F32, tag="b2" + b2d.name)
                nc.sync.dma_start(b2[:], b2d[:])
                ps1 = pip.tile([128, 8, B], F32, tag="ps1")
                for mt in range(8):
                    for kt in range(16):
                        nc.tensor.matmul(ps1[:, mt, :], w1[:, kt, mt, :],
                                         img_sb[:, kt, :],
                                         start=(kt == 0), stop=(kt == 15))
                h1 = ewp.tile([128, 8, B], BF16, tag="h1mlp")
                for mt in range(8):
                    nc.scalar.activation(h1[:, mt, :], ps1[:, mt, :], AF.Relu,
                                         bias=b1[:, mt:mt + 1])
                ps2 = pip.tile([128, 4, B], F32, tag="ps2")
                for mt in range(4):
                    for kt in range(8):
                        nc.tensor.matmul(ps2[:, mt, :], w2[:, kt, mt, :],
                                         h1[:, kt, :],
                                         start=(kt == 0), stop=(kt == 7))
                for mt in range(4):
                    nc.scalar.activation(out_ap[:, mt, :], ps2[:, mt, :],
                                         AF.Relu, bias=b2[:, mt:mt + 1])

            init_mlp(Wh1, bh1, Wh2, bh2, hT0, BF16)
            init_mlp(Wc1, bc1, Wc2, bc2, cT, F32)

            # ---- interleaved per-call input projection machinery
            call_xg = {}

            def proj_closures(k):
                """Returns emission closures: x DMA + 16 m-tile projections."""
                L = LS[k]
                st = {}

                def start():
                    x_sb = xp.tile([128, KX, L * B], BF16, tag="x")
                    nc.sync.dma_start(
                        x_sb[:], xt[:, :, POS0[k] * B:(POS0[k] + L) * B])
                    xg_sb = xgp.tile([128, 16, L, B], F32, tag="xg")
                    st["x"] = x_sb
                    call_xg[k] = xg_sb

                def m_op(m):
                    x_sb = st["x"]
                    xg_sb = call_xg[k]
                    pp = ppp.tile([128, L * B], F32, tag="pp")
                    for kt in range(KX):
                        nc.tensor.matmul(pp[:], wih_sb[:, kt, m, :],
                                         x_sb[:, kt, :],
                                         start=(kt == 0), stop=(kt == KX - 1))
                    nc.scalar.activation(
                        xg_sb[:, m].rearrange("p l b -> p (l b)"), pp[:],
                        AF.Identity, bias=bg_sb[:, m:m + 1])

                return [start] + [
                    (lambda m=m: m_op(m)) for m in range(16)]

            from collections import deque
            pq = deque()
            for c in proj_closures(0):
                c()
            for c in proj_closures(1):
                c()

            # ---- the chain
            prev_h = None  # (tile, L) of previous call
            for k in range(len(CALLS)):
                L = LS[k]
                if k + 2 < len(CALLS):
                    pq.extend(proj_closures(k + 2))
                xg_sb = call_xg.pop(k)
                xgv = xg_sb.rearrange("p (g j) l b -> p g j l b", g=4)
                h_sb = hp.tile([128, 4, L, B], BF16, tag="h")
                # proj pop rate: drain queue over this call's steps
                rate = max(1, -(-len(pq) // max(1, 2 * L)))

                for s in range(L):
                    if s == 0:
                        if prev_h is None:
                            hsrc = lambda kt: hT0[:, kt, :]
                        else:
                            ph, pL = prev_h
                            hsrc = lambda kt, ph=ph, pL=pL: ph[:, kt, pL - 1, :]
                    else:
                        hsrc = lambda kt, s=s: h_sb[:, kt, s - 1, :]

                    # two psum tiles (separate banks): k-halves accumulate
                    # independently; groups within a bank stay consecutive
                    # (start=True clears has_written bank-wide).
                    pgA = pgp.tile([128, 4, 4, B], F32, tag="pgA")
                    pgB = pgp.tile([128, 4, 4, B], F32, tag="pgB")
                    for (jlo, jhi), kts in (((0, 2), (0, 2)), ((0, 2), (2, 4)),
                                            ((2, 4), (0, 2)), ((2, 4), (2, 4))):
                        pg_ = pgA if kts[0] == 0 else pgB
                        for g in range(4):
                            for jj in range(jlo, jhi):
                                for kt in range(*kts):
                                    nc.tensor.matmul(
                                        pg_[:, g, jj, :],
                                        whh_sb[:, kt, g * 4 + jj, :],
                                        hsrc(kt),
                                        start=(kt % 2 == 0), stop=(kt % 2 == 1),
                                        skip_group_check=True)
                    # elementwise, split in two j-halves
                    g_sb = ewp.tile([128, 4, 4, B], F32, tag="g")
                    s_sb = ewp.tile([128, 3, 4, B], F32, tag="s")
                    tg = ewp.tile([128, 4, B], F32, tag="tg")
                    tc_ = ewp.tile([128, 4, B], F32, tag="tc")
                    tmp = ewp.tile([128, 4, B], F32, tag="tmp")
                    for jh in (0, 1):
                        ch = slice(2 * jh, 2 * jh + 2)
                        nc.vector.tensor_tensor(
                            g_sb[:, :, ch, :], pgA[:, :, ch, :],
                            xgv[:, :, ch, s, :], ALU.add)
                        nc.vector.tensor_tensor(
                            g_sb[:, :, ch, :], pgB[:, :, ch, :],
                            g_sb[:, :, ch, :], ALU.add)
                        nc.scalar.activation(
                            s_sb[:, :, ch, :], g_sb[:, 0:3, ch, :], AF.Sigmoid)
                        nc.scalar.activation(
                            tg[:, ch, :], g_sb[:, 3, ch, :], AF.Tanh)
                        nc.vector.tensor_tensor(
                            tmp[:, ch, :], s_sb[:, 0, ch, :], tg[:, ch, :],
                            ALU.mult)
                        nc.vector.tensor_tensor(
                            cT[:, ch, :], s_sb[:, 1, ch, :], cT[:, ch, :],
                            ALU.mult)
                        nc.vector.tensor_tensor(
                            cT[:, ch, :], cT[:, ch, :], tmp[:, ch, :], ALU.add)
                        nc.scalar.activation(
                            tc_[:, ch, :], cT[:, ch, :], AF.Tanh)
                        nc.vector.tensor_tensor(
                            h_sb[:, ch, s, :], s_sb[:, 2, ch, :],
                            tc_[:, ch, :], ALU.mult)
                    for _ in range(2 * rate):
                        if pq:
                            pq.popleft()()
                nc.sync.dma_start(oh[:, :, POS0[k]:POS0[k] + L, :],
                                  h_sb[:, :, 0:L, :])
                prev_h = (h_sb, L)
            while pq:
                pq.popleft()()
    nc.compile()
    return nc


def build_fc():
    """FC head NEFF: logits[r, v] = y[r] @ Wfc[:, vshard] + bfc, per core."""
    nc = bacc.Bacc()
    yT = nc.dram_tensor("yT", [128, 8, RPAD], BF16, kind="ExternalInput")
    Wfc = nc.dram_tensor("Wfct", [128, 8, VL], BF16, kind="ExternalInput")
    bfc = nc.dram_tensor("bfcr", [128, VL], F32, kind="ExternalInput")
    out = nc.dram_tensor("logits", [RPAD, VL], F32, kind="ExternalOutput")
    with tile.TileContext(nc) as tc:
        with (
            tc.tile_pool(name="const", bufs=1) as cp,
            tc.tile_pool(name="ob", bufs=4) as op,
            tc.tile_pool(name="ps", bufs=4, space="PSUM") as pp,
        ):
            y_sb = cp.tile([128, 8, RPAD], BF16)
            nc.sync.dma_start(y_sb[:], yT[:])
            w_sb = cp.tile([128, 8, VL], BF16)
            nc.sync.dma_start(w_sb[:], Wfc[:])
            b_sb = cp.tile([128, VL], F32)
            nc.sync.dma_start(b_sb[:], bfc[:])
            chunks = [(c0, min(512, VL - c0)) for c0 in range(0, VL, 512)]
            for mt in range(RPAD // 128):
                for (c0, cs) in chunks:
                    ps = pp.tile([128, 512], F32, tag="ps")
                    for kt in range(8):
                        nc.tensor.matmul(
                            ps[:, :cs], y_sb[:, kt, mt * 128:(mt + 1) * 128],
                            w_sb[:, kt, c0:c0 + cs],
                            start=(kt == 0), stop=(kt == 7))
                    o_sb = op.tile([128, 512], F32, tag="o")
                    nc.vector.tensor_tensor(o_sb[:, :cs], ps[:, :cs],
                                            b_sb[:, c0:c0 + cs], ALU.add)
                    nc.sync.dma_start(
                        out[mt * 128:(mt + 1) * 128, c0:c0 + cs], o_sb[:, :cs])
    nc.compile()
    return nc


# ---------------------------------------------------------------- runner

_CACHE = {}


class _Runner:
    """Compile a Bacc module once into a sharded PJRT executable over the 8
    cores; allow warm re-execution for timing (device-resident inputs)."""

    def __init__(self, nc):
        import jax
        from jax.sharding import Mesh, PartitionSpec, NamedSharding
        from jax.experimental.shard_map import shard_map
        from concourse import bass2jax, mybir as _mb
        bass2jax.install_neuronx_cc_hook()
        self.jax = jax
        self.nc = nc
        partition_name = (nc.partition_id_tensor.name
                          if nc.partition_id_tensor else None)
        in_names, out_names, out_avals, zero_outs = [], [], [], []
        self.in_specs = {}
        for alloc in nc.m.functions[0].allocations:
            if not isinstance(alloc, _mb.MemoryLocationSet):
                continue
            name = alloc.memorylocations[0].name
            if alloc.kind == "ExternalInput":
                if name != partition_name:
                    in_names.append(name)
                    self.in_specs[name] = (tuple(alloc.tensor_shape),
                                           _mb.dt.np(alloc.dtype))
            elif alloc.kind == "ExternalOutput":
                shape = tuple(alloc.tensor_shape)
                dtype = _mb.dt.np(alloc.dtype)
                out_names.append(name)
                out_avals.append(jax.core.ShapedArray(shape, dtype))
                zero_outs.append(np.zeros(shape, dtype))
        self.in_names = list(in_names)
        self.out_names = out_names
        self.out_avals = out_avals
        self.zero_outs = zero_outs
        n_params = len(in_names)
        all_in = in_names + out_names
        if partition_name is not None:
            all_in.append(partition_name)

        def _body(*args):
            operands = list(args)
            if partition_name is not None:
                operands.append(bass2jax.partition_id_tensor())
            return tuple(bass2jax._bass_exec_p.bind(
                *operands,
                out_avals=tuple(out_avals),
                in_names=tuple(all_in),
                out_names=tuple(out_names),
                lowering_input_output_aliases=(),
                sim_require_finite=True,
                sim_require_nnan=True,
                nc=nc,
            ))

        devices = jax.devices()[:NCORES]
        self.mesh = Mesh(np.asarray(devices), ("core",))
        self.sharding = NamedSharding(self.mesh, PartitionSpec("core"))
        n_in = n_params + len(out_names)
        self.sharded = jax.jit(shard_map(
            _body, mesh=self.mesh,
            in_specs=(PartitionSpec("core"),) * n_in,
            out_specs=(PartitionSpec("core"),) * len(out_names),
            check_rep=False), keep_unused=True)
        self._zeros_dev = None

    def warm(self):
        """trigger jit trace + neuronx compile with zero inputs."""
        zmap = {n: np.zeros(s, d) for n, (s, d) in self.in_specs.items()}
        self.run([zmap] * NCORES)

    def stage(self, in_maps):
        """host->device transfer of per-core inputs; returns device args."""
        jax = self.jax
        concat = [np.concatenate([np.asarray(m[n]) for m in in_maps], axis=0)
                  for n in self.in_names]
        args = [jax.device_put(a, self.sharding) for a in concat]
        if self._zeros_dev is None:
            self._zeros_dev = [
                jax.device_put(
                    np.zeros((NCORES * z.shape[0], *z.shape[1:]), z.dtype),
                    self.sharding) for z in self.zero_outs]
        args += self._zeros_dev
        for a in args:
            a.block_until_ready()
        return args

    def execute(self, args):
        outs = self.sharded(*args)
        for o in outs:
            o.block_until_ready()
        return outs

    def burst(self, args, reps=16, tries=3):
        """min total seconds for `reps` pipelined dispatches (async submit,
        block once at the end) — marginal per-exec isolates device time from
        the fixed dispatch floor."""
        import time as _t
        self.execute(args)  # warm
        best = float("inf")
        for _ in range(tries):
            t0 = _t.perf_counter()
            outs = None
            for _ in range(reps):
                outs = self.sharded(*args)
            for o in outs:
                o.block_until_ready()
            best = min(best, _t.perf_counter() - t0)
        return best / reps

    def run(self, in_maps, time_reps=0):
        args = self.stage(in_maps)
        outs = self.execute(args)  # cold (compiles first time)
        if time_reps:
            _run.times.append(int(self.burst(args) * 1e9))
        res = []
        for c in range(NCORES):
            res.append({
                name: np.asarray(outs[i]).reshape(
                    NCORES, *self.out_avals[i].shape)[c]
                for i, name in enumerate(self.out_names)})
        return res


import threading as _threading
_CACHE_LOCKS = {k: _threading.Lock() for k in (2, 8, "fc")}


def _get_nc(key):
    with _CACHE_LOCKS[key]:
        if key not in _CACHE:
            nc = build_fc() if key == "fc" else build_chain(key)
            _CACHE[key] = _Runner(nc)
    return _CACHE[key]


def _run(runner, in_maps, trace=False):
    return runner.run(in_maps, time_reps=3 if trace else 0)


_run.times = []


def _fc_shards(inp):
    Wfc = inp["Wfc"].astype(np.float32)
    bfc = inp["bfc"].astype(np.float32)
    shards = []
    for c in range(NCORES):
        v0 = c * VL
        wt = np.ascontiguousarray(
            Wfc[:, v0:v0 + VL].reshape(8, 128, VL).transpose(1, 0, 2)
        ).astype(nbf)
        bt = np.broadcast_to(bfc[v0:v0 + VL], (128, VL)).copy()
        shards.append((wt, bt))
    return shards


def kernel(**inputs):
    trace = bool(int(os.environ.get("CAPNET_TRACE", "0")))
    _run.times = []
    inp = {k: np.asarray(v) for k, v in inputs.items()}
    return _kernel_3phase(inp, trace)


def _kernel_3phase(inp, trace):
    per_dir = _chain_host_inputs(inp)

    # ---- phase 1: layer-0 chains (core 0 fwd, core 1 bwd)
    nc0 = _get_nc(2)
    maps0 = []
    for c in range(NCORES):
        d = c % 2
        m = {k: per_dir[d][k] for k in ("imgT", "Wh1t", "bh1t", "Wh2t", "bh2t",
                                        "Wc1t", "bc1t", "Wc2t", "bc2t")}
        m["Whh"] = per_dir[d]["Whh0"]
        m["Wih"] = per_dir[d]["Wih0"]
        m["bg"] = per_dir[d]["bg0"]
        maps0.append(m)
    x0f = _x0_arranged(inp, rev=False)
    x0b = _x0_arranged(inp, rev=True)
    for c in range(NCORES):
        maps0[c]["xt"] = x0f if c % 2 == 0 else x0b
    res0 = _run(nc0, maps0, trace=trace)
    h0f = _oh_to_HposB(res0[0]["oh"])
    h0b = _oh_to_HposB(res0[1]["oh"])

    # ---- phase 2: layer-1 chains
    nc1 = _get_nc(8)
    maps1 = []
    for c in range(NCORES):
        d = c % 2
        m = {k: per_dir[d][k] for k in ("imgT", "Wh1t", "bh1t", "Wh2t", "bh2t",
                                        "Wc1t", "bc1t", "Wc2t", "bc2t")}
        m["Whh"] = per_dir[d]["Whh1"]
        m["Wih"] = per_dir[d]["Wih1"]
        m["bg"] = per_dir[d]["bg1"]
        maps1.append(m)
    x1f = _x1_arranged(h0f, h0b, rev=False)
    x1b = _x1_arranged(h0f, h0b, rev=True)
    for c in range(NCORES):
        maps1[c]["xt"] = x1f if c % 2 == 0 else x1b
    res1 = _run(nc1, maps1, trace=trace)
    h1f = _oh_to_HposB(res1[0]["oh"])
    h1b = _oh_to_HposB(res1[1]["oh"])

    # ---- phase 3: FC head (vocab-sharded)
    ncf = _get_nc("fc")
    yT = _y_assemble(h1f, h1b)
    fcs = _fc_shards(inp)
    mapsf = [{"yT": yT, "Wfct": fcs[c][0], "bfcr": fcs[c][1]}
             for c in range(NCORES)]
    resf = _run(ncf, mapsf, trace=trace)

    logits = np.empty((N, T, B, V), np.float32)
    for c in range(NCORES):
        logits[:, :, :, c * VL:(c + 1) * VL] = (
            resf[c]["logits"][:800].reshape(N, T, B, VL))
    return logits


# revision 24
# speedup vs baseline: 89.9081x; 1.1391x over previous
"""Trainium2 Bass kernel for nn_CaptionNet_23467701305971.

Model: image-captioning net. init MLPs -> 2-layer biLSTM with a redundant
prefix-recomputation state chain (50 sequential calls, 275 LSTM steps per
direction-chain) -> big FC head to vocab 30000.

Strategy (8 NeuronCores):
  - The 4 direction-chains (l0f, l0b, l1f, l1b) are strictly sequential
    inside, but l0f/l0b are independent and l1f/l1b depend on l0 outputs.
  - Phase 1: chain NEFF (one SPMD program, role differences are pure DATA):
    core 0 runs the layer-0 forward chain, core 1 the layer-0 backward chain
    (backward = same program on time-reversed per-call inputs).
  - Host glue: assemble layer-1 inputs x1 = concat(of, ob) in consumption
    order per direction.
  - Phase 2: same chain program (wider input dim) runs layer-1 fwd/bwd on
    cores 0/1.
  - Phase 3: FC head, vocab-sharded across all 8 cores.
  - All matmuls bf16 with fp32 PSUM accumulation; cell state c and gate
    pre-activations stay fp32.  Measured numeric error vs the fp32
    reference: ~4e-3 relative L2.

Kernel layout notes:
  - Everything is "transposed": H lives on SBUF partitions. The recurrent
    matmul is weight-stationary: 64 (LDW+MM) pairs of [128k x 128m] @ [128k,
    16batch] per step, gates land on partitions so sigmoid/tanh run on 128
    lanes.
  - Gate order is host-permuted to (i, f, o, g) so one ACT op covers all
    sigmoids.
  - The per-call input projection (xg = x @ Wih + b) is emitted interleaved
    with chain steps two calls ahead, filling PE bubbles left by the
    elementwise chain.
"""

import os
import sys
import numpy as np
import ml_dtypes

sys.path.insert(0, "/opt/trn_rl_repo")

import concourse.bass as bass  # noqa: E402
from concourse import bacc  # noqa: E402
import concourse.tile as tile  # noqa: E402
import concourse.mybir as mybir  # noqa: E402

BF16 = mybir.dt.bfloat16
F32 = mybir.dt.float32
AF = mybir.ActivationFunctionType
ALU = mybir.AluOpType

B, N, T, H, E, V, F = 16, 5, 10, 512, 250, 30000, 2048
CALLS = [(t, n) for t in range(T) for n in range(N)]
LS = [t + 1 for (t, n) in CALLS]
POS0 = np.concatenate([[0], np.cumsum(LS)]).astype(int)
NPOS = int(POS0[-1])  # 275
NCORES = 8
VL = V // NCORES  # 3750
RPAD = 896  # 800 output rows padded to 7*128

nbf = ml_dtypes.bfloat16


# ---------------------------------------------------------------- host prep

def _perm_gates(W):
    """reorder gate blocks (i,f,g,o) -> (i,f,o,g) along the last axis."""
    Hh = W.shape[-1] // 4
    return np.concatenate(
        [W[..., :Hh], W[..., Hh:2 * Hh], W[..., 3 * Hh:], W[..., 2 * Hh:3 * Hh]],
        axis=-1)


def _tile_w(W, KX, MT):
    """[Din, MT*128] -> [128, KX, MT, 128] bf16 stationary tiles."""
    Din, M = W.shape
    assert M == MT * 128
    Wp = np.zeros((KX * 128, M), np.float32)
    Wp[:Din] = W
    return np.ascontiguousarray(
        Wp.reshape(KX, 128, MT, 128).transpose(1, 0, 2, 3)).astype(nbf)


def _tile_b(b, MT):
    return np.ascontiguousarray(b.reshape(MT, 128).T).astype(np.float32)


def _chain_host_inputs(inp):
    """Per-core input dicts for the two chain phases (minus the x inputs)."""
    com = {
        "imgT": np.ascontiguousarray(
            inp["img"].T.reshape(16, 128, B).transpose(1, 0, 2)).astype(nbf),
        "Wh1t": _tile_w(inp["Wh1"], 16, 8), "bh1t": _tile_b(inp["bh1"], 8),
        "Wh2t": _tile_w(inp["Wh2"], 8, 4), "bh2t": _tile_b(inp["bh2"], 4),
        "Wc1t": _tile_w(inp["Wc1"], 16, 8), "bc1t": _tile_b(inp["bc1"], 8),
        "Wc2t": _tile_w(inp["Wc2"], 8, 4), "bc2t": _tile_b(inp["bc2"], 4),
    }
    per_dir = {}
    for d, sfx in ((0, "f"), (1, "b")):
        per_dir[d] = dict(com)
    for d, sfx in ((0, "f"), (1, "b")):
        per_dir[d]["Whh0"] = _tile_w(_perm_gates(inp["Whh0" + sfx]), 4, 16)
        per_dir[d]["Wih0"] = _tile_w(_perm_gates(inp["Wih0" + sfx]), 2, 16)
        per_dir[d]["bg0"] = _tile_b(_perm_gates(inp["b0" + sfx]), 16)
        per_dir[d]["Whh1"] = _tile_w(_perm_gates(inp["Whh1" + sfx]), 4, 16)
        per_dir[d]["Wih1"] = _tile_w(_perm_gates(inp["Wih1" + sfx]), 8, 16)
        per_dir[d]["bg1"] = _tile_b(_perm_gates(inp["b1" + sfx]), 16)
    return per_dir


def _x0_arranged(inp, rev):
    """layer-0 chain input, consumption order, transposed: [128, 2, NPOS*B]."""
    seq = inp["emb"][inp["caps"]].transpose(1, 2, 0, 3)  # [N, T, B, E]
    A = np.zeros((NPOS, B, 256), np.float32)
    for k, (t, n) in enumerate(CALLS):
        L = t + 1
        for s in range(L):
            tok = (L - 1 - s) if rev else s
            A[POS0[k] + s] = np.pad(seq[n, tok], ((0, 0), (0, 6)))
    return np.ascontiguousarray(
        A.reshape(NPOS * B, 2, 128).transpose(2, 1, 0)).astype(nbf)


def _oh_to_HposB(oh):
    """device oh [128, 4, NPOS, B] -> [H, NPOS, B] float32."""
    return oh.astype(np.float32).transpose(1, 0, 2, 3).reshape(H, NPOS, B)


def _x1_arranged(hf, hb, rev):
    """layer-1 chain input [128, 8, NPOS*B] bf16 from layer-0 outputs.

    hf/hb: [H, NPOS, B] layer-0 fwd/bwd chain outputs in their own
    consumption order (fwd slot s = natural s; bwd slot s = natural L-1-s).
    """
    pf = np.zeros(NPOS, int)
    pb = np.zeros(NPOS, int)
    for k, (t, n) in enumerate(CALLS):
        L = t + 1
        for s in range(L):  # s = consumption slot of the l1 chain
            nat = (L - 1 - s) if rev else s  # natural time of this slot
            pf[POS0[k] + s] = POS0[k] + nat          # fwd chain slot = nat
            pb[POS0[k] + s] = POS0[k] + (L - 1 - nat)  # bwd chain slot
    A = np.concatenate([hf[:, pf, :], hb[:, pb, :]], axis=0)  # [1024, NPOS, B]
    return np.ascontiguousarray(
        A.reshape(8, 128, NPOS * B).transpose(1, 0, 2)).astype(nbf)


def _y_assemble(h1f, h1b):
    """final FC input yT [128, 8, RPAD] bf16 from layer-1 chain outputs."""
    y = np.zeros((2 * H, RPAD), np.float32)
    for n in range(N):
        k = 45 + n
        L = 10
        for s in range(L):
            r = (n * T + s) * B
            y[:H, r:r + B] = h1f[:, POS0[k] + s, :]
            y[H:, r:r + B] = h1b[:, POS0[k] + L - 1 - s, :]
    return np.ascontiguousarray(
        y.reshape(8, 128, RPAD).transpose(1, 0, 2)).astype(nbf)


# ---------------------------------------------------------------- builders

def build_chain(KX):
    """Chain NEFF. KX = input k-tiles (2 for layer-0, 8 for layer-1).

    Inputs (per core): imgT, W/b for both init MLPs, xt [128,KX,NPOS*B] bf16
    (arranged consumption-order rows, transposed), Wih [128,KX,16,128],
    bg [128,16], Whh [128,4,16,128].
    Output: oh [128, 4, NPOS, B] bf16 (per-slot hidden states).
    """
    nc = bacc.Bacc()
    imgT = nc.dram_tensor("imgT", [128, 16, B], BF16, kind="ExternalInput")
    Wh1 = nc.dram_tensor("Wh1t", [128, 16, 8, 128], BF16, kind="ExternalInput")
    bh1 = nc.dram_tensor("bh1t", [128, 8], F32, kind="ExternalInput")
    Wh2 = nc.dram_tensor("Wh2t", [128, 8, 4, 128], BF16, kind="ExternalInput")
    bh2 = nc.dram_tensor("bh2t", [128, 4], F32, kind="ExternalInput")
    Wc1 = nc.dram_tensor("Wc1t", [128, 16, 8, 128], BF16, kind="ExternalInput")
    bc1 = nc.dram_tensor("bc1t", [128, 8], F32, kind="ExternalInput")
    Wc2 = nc.dram_tensor("Wc2t", [128, 8, 4, 128], BF16, kind="ExternalInput")
    bc2 = nc.dram_tensor("bc2t", [128, 4], F32, kind="ExternalInput")
    xt = nc.dram_tensor("xt", [128, KX, NPOS * B], BF16, kind="ExternalInput")
    Wih = nc.dram_tensor("Wih", [128, KX, 16, 128], BF16, kind="ExternalInput")
    bg = nc.dram_tensor("bg", [128, 16], F32, kind="ExternalInput")
    Whh = nc.dram_tensor("Whh", [128, 4, 16, 128], BF16, kind="ExternalInput")
    oh = nc.dram_tensor("oh", [128, 4, NPOS, B], BF16, kind="ExternalOutput")

    with tile.TileContext(nc) as tc:
        with (
            tc.tile_pool(name="const", bufs=1) as cp,
            tc.tile_pool(name="xp", bufs=3) as xp,
            tc.tile_pool(name="xgp", bufs=3) as xgp,
            tc.tile_pool(name="hp", bufs=2) as hp,
            tc.tile_pool(name="ewp", bufs=2) as ewp,
            tc.tile_pool(name="sp", bufs=1) as sp,
            tc.tile_pool(name="pgp", bufs=2, space="PSUM") as pgp,
            tc.tile_pool(name="ppp", bufs=2, space="PSUM") as ppp,
            tc.tile_pool(name="pip", bufs=1, space="PSUM") as pip,
        ):
            # ---- load weights
            img_sb = cp.tile([128, 16, B], BF16)
            nc.sync.dma_start(img_sb[:], imgT[:])
            whh_sb = cp.tile([128, 4, 16, 128], BF16)
            nc.sync.dma_start(whh_sb[:], Whh[:])
            wih_sb = cp.tile([128, KX, 16, 128], BF16)
            nc.sync.dma_start(wih_sb[:], Wih[:])
            bg_sb = cp.tile([128, 16], F32)
            nc.sync.dma_start(bg_sb[:], bg[:])

            # ---- init MLPs -> hT0 (bf16) / cA,cB (f32) [128, 2, B] halves
            # (per-half state tiles avoid any false cross-half serialization)
            cA = sp.tile([128, 2, B], F32)   # persistent cell state j 0..1
            cB = sp.tile([128, 2, B], F32)   # persistent cell state j 2..3
            hT0 = sp.tile([128, 4, B], BF16)

            def init_mlp(W1d, b1d, W2d, b2d, out_fn):
                w1 = cp.tile([128, 16, 8, 128], BF16, tag="w1" + W1d.name)
                nc.sync.dma_start(w1[:], W1d[:])
                b1 = cp.tile([128, 8], F32, tag="b1" + b1d.name)
                nc.sync.dma_start(b1[:], b1d[:])
                w2 = cp.tile([128, 8, 4, 128], BF16, tag="w2" + W2d.name)
                nc.sync.dma_start(w2[:], W2d[:])
                b2 = cp.tile([128, 4], F32, tag="b2" + b2d.name)
                nc.sync.dma_start(b2[:], b2d[:])
                ps1 = pip.tile([128, 8, B], F32, tag="ps1")
                for mt in range(8):
                    for kt in range(16):
                        nc.tensor.matmul(ps1[:, mt, :], w1[:, kt, mt, :],
                                         img_sb[:, kt, :],
                                         start=(kt == 0), stop=(kt == 15))
                h1 = ewp.tile([128, 8, B], BF16, tag="h1mlp")
                for mt in range(8):
                    nc.scalar.activation(h1[:, mt, :], ps1[:, mt, :], AF.Relu,
                                         bias=b1[:, mt:mt + 1])
                ps2 = pip.tile([128, 4, B], F32, tag="ps2")
                for mt in range(4):
                    for kt in range(8):
                        nc.tensor.matmul(ps2[:, mt, :], w2[:, kt, mt, :],
                                         h1[:, kt, :],
                                         start=(kt == 0), stop=(kt == 7))
                for mt in range(4):
                    nc.scalar.activation(out_fn(mt), ps2[:, mt, :],
                                         AF.Relu, bias=b2[:, mt:mt + 1])

            init_mlp(Wh1, bh1, Wh2, bh2, lambda mt: hT0[:, mt, :])
            init_mlp(Wc1, bc1, Wc2, bc2,
                     lambda mt: (cA if mt < 2 else cB)[:, mt % 2, :])

            # ---- interleaved per-call input projection machinery
            call_xg = {}

            def proj_closures(k):
                """Returns emission closures: x DMA + 16 m-tile projections."""
                L = LS[k]
                st = {}

                def start():
                    x_sb = xp.tile([128, KX, L * B], BF16, tag="x")
                    nc.sync.dma_start(
                        x_sb[:], xt[:, :, POS0[k] * B:(POS0[k] + L) * B])
                    xg_sb = xgp.tile([128, 16, L, B], F32, tag="xg")
                    st["x"] = x_sb
                    call_xg[k] = xg_sb

                def m_op(m):
                    x_sb = st["x"]
                    xg_sb = call_xg[k]
                    pp = ppp.tile([128, L * B], F32, tag="pp")
                    for kt in range(KX):
                        nc.tensor.matmul(pp[:], wih_sb[:, kt, m, :],
                                         x_sb[:, kt, :],
                                         start=(kt == 0), stop=(kt == KX - 1))
                    nc.scalar.activation(
                        xg_sb[:, m].rearrange("p l b -> p (l b)"), pp[:],
                        AF.Identity, bias=bg_sb[:, m:m + 1])

                return [start] + [
                    (lambda m=m: m_op(m)) for m in range(16)]

            from collections import deque
            pq = deque()
            for c in proj_closures(0):
                c()
            for c in proj_closures(1):
                c()

            # ---- the chain
            prev_h = None  # (tile, L) of previous call
            for k in range(len(CALLS)):
                L = LS[k]
                if k + 2 < len(CALLS):
                    pq.extend(proj_closures(k + 2))
                xg_sb = call_xg.pop(k)
                xgv = xg_sb.rearrange("p (g j) l b -> p g j l b", g=4)
                hA_sb = hp.tile([128, 2, L, B], BF16, tag="hA")
                hB_sb = hp.tile([128, 2, L, B], BF16, tag="hB")
                # proj pop rate: drain queue over this call's steps
                rate = max(1, -(-len(pq) // max(1, 2 * L)))

                for s in range(L):
                    if s == 0:
                        if prev_h is None:
                            hsrc = lambda kt: hT0[:, kt, :]
                        else:
                            pa, pb_, pL = prev_h
                            hsrc = (lambda kt, pa=pa, pb_=pb_, pL=pL:
                                    (pa if kt < 2 else pb_)[:, kt % 2, pL - 1, :])
                    else:
                        hsrc = (lambda kt, s=s:
                                (hA_sb if kt < 2 else hB_sb)[:, kt % 2, s - 1, :])

                    # two psum tiles (separate banks): k-halves accumulate
                    # independently; groups within a bank stay consecutive
                    # (start=True clears has_written bank-wide).
                    pgA = pgp.tile([128, 4, 4, B], F32, tag="pgA")
                    pgB = pgp.tile([128, 4, 4, B], F32, tag="pgB")
                    for (jlo, jhi), kts in (((0, 2), (0, 2)), ((0, 2), (2, 4)),
                                            ((2, 4), (0, 2)), ((2, 4), (2, 4))):
                        pg_ = pgA if kts[0] == 0 else pgB
                        for g in range(4):
                            for jj in range(jlo, jhi):
                                for kt in range(*kts):
                                    nc.tensor.matmul(
                                        pg_[:, g, jj, :],
                                        whh_sb[:, kt, g * 4 + jj, :],
                                        hsrc(kt),
                                        start=(kt % 2 == 0), stop=(kt % 2 == 1),
                                        skip_group_check=True)
                    # elementwise, split in two j-halves with dedicated
                    # tiles per half (no shared-tile false dependencies)
                    for jh in (0, 1):
                        ch = slice(2 * jh, 2 * jh + 2)
                        cH = cA if jh == 0 else cB
                        hH = hA_sb if jh == 0 else hB_sb
                        gh = ewp.tile([128, 4, 2, B], F32, tag=f"g{jh}",
                                      name=f"g{jh}")
                        sh = ewp.tile([128, 3, 2, B], F32, tag=f"s{jh}",
                                      name=f"s{jh}")
                        tgh = ewp.tile([128, 2, B], F32, tag=f"tg{jh}",
                                       name=f"tg{jh}")
                        tch = ewp.tile([128, 2, B], F32, tag=f"tc{jh}",
                                       name=f"tc{jh}")
                        tmph = ewp.tile([128, 2, B], F32, tag=f"tmp{jh}",
                                        name=f"tmp{jh}")
                        nc.vector.tensor_tensor(
                            gh[:], pgA[:, :, ch, :], xgv[:, :, ch, s, :],
                            ALU.add)
                        nc.vector.tensor_tensor(
                            gh[:], pgB[:, :, ch, :], gh[:], ALU.add)
                        nc.scalar.activation(sh[:], gh[:, 0:3], AF.Sigmoid)
                        nc.scalar.activation(tgh[:], gh[:, 3], AF.Tanh)
                        nc.vector.tensor_tensor(
                            tmph[:], sh[:, 0], tgh[:], ALU.mult)
                        nc.vector.tensor_tensor(
                            cH[:], sh[:, 1], cH[:], ALU.mult)
                        nc.vector.tensor_tensor(
                            cH[:], cH[:], tmph[:], ALU.add)
                        nc.scalar.activation(tch[:], cH[:], AF.Tanh)
                        nc.vector.tensor_tensor(
                            hH[:, :, s, :], sh[:, 2], tch[:], ALU.mult)
                    for _ in range(2 * rate):
                        if pq:
                            pq.popleft()()
                nc.sync.dma_start(oh[:, 0:2, POS0[k]:POS0[k] + L, :],
                                  hA_sb[:, :, 0:L, :])
                nc.sync.dma_start(oh[:, 2:4, POS0[k]:POS0[k] + L, :],
                                  hB_sb[:, :, 0:L, :])
                prev_h = (hA_sb, hB_sb, L)
            while pq:
                pq.popleft()()
    nc.compile()
    return nc


def build_fc():
    """FC head NEFF: logits[r, v] = y[r] @ Wfc[:, vshard] + bfc, per core."""
    nc = bacc.Bacc()
    yT = nc.dram_tensor("yT", [128, 8, RPAD], BF16, kind="ExternalInput")
    Wfc = nc.dram_tensor("Wfct", [128, 8, VL], BF16, kind="ExternalInput")
    bfc = nc.dram_tensor("bfcr", [128, VL], F32, kind="ExternalInput")
    out = nc.dram_tensor("logits", [RPAD, VL], F32, kind="ExternalOutput")
    with tile.TileContext(nc) as tc:
        with (
            tc.tile_pool(name="const", bufs=1) as cp,
            tc.tile_pool(name="ob", bufs=4) as op,
            tc.tile_pool(name="ps", bufs=4, space="PSUM") as pp,
        ):
            y_sb = cp.tile([128, 8, RPAD], BF16)
            nc.sync.dma_start(y_sb[:], yT[:])
            w_sb = cp.tile([128, 8, VL], BF16)
            nc.sync.dma_start(w_sb[:], Wfc[:])
            b_sb = cp.tile([128, VL], F32)
            nc.sync.dma_start(b_sb[:], bfc[:])
            chunks = [(c0, min(512, VL - c0)) for c0 in range(0, VL, 512)]
            for mt in range(RPAD // 128):
                for (c0, cs) in chunks:
                    ps = pp.tile([128, 512], F32, tag="ps")
                    for kt in range(8):
                        nc.tensor.matmul(
                            ps[:, :cs], y_sb[:, kt, mt * 128:(mt + 1) * 128],
                            w_sb[:, kt, c0:c0 + cs],
                            start=(kt == 0), stop=(kt == 7))
                    o_sb = op.tile([128, 512], F32, tag="o")
                    nc.vector.tensor_tensor(o_sb[:, :cs], ps[:, :cs],
                                            b_sb[:, c0:c0 + cs], ALU.add)
                    nc.sync.dma_start(
                        out[mt * 128:(mt + 1) * 128, c0:c0 + cs], o_sb[:, :cs])
    nc.compile()
    return nc


# ---------------------------------------------------------------- runner

_CACHE = {}


class _Runner:
    """Compile a Bacc module once into a sharded PJRT executable over the 8
    cores; allow warm re-execution for timing (device-resident inputs)."""

    def __init__(self, nc):
        import jax
        from jax.sharding import Mesh, PartitionSpec, NamedSharding
        from jax.experimental.shard_map import shard_map
        from concourse import bass2jax, mybir as _mb
        bass2jax.install_neuronx_cc_hook()
        self.jax = jax
        self.nc = nc
        partition_name = (nc.partition_id_tensor.name
                          if nc.partition_id_tensor else None)
        in_names, out_names, out_avals, zero_outs = [], [], [], []
        self.in_specs = {}
        for alloc in nc.m.functions[0].allocations:
            if not isinstance(alloc, _mb.MemoryLocationSet):
                continue
            name = alloc.memorylocations[0].name
            if alloc.kind == "ExternalInput":
                if name != partition_name:
                    in_names.append(name)
                    self.in_specs[name] = (tuple(alloc.tensor_shape),
                                           _mb.dt.np(alloc.dtype))
            elif alloc.kind == "ExternalOutput":
                shape = tuple(alloc.tensor_shape)
                dtype = _mb.dt.np(alloc.dtype)
                out_names.append(name)
                out_avals.append(jax.core.ShapedArray(shape, dtype))
                zero_outs.append(np.zeros(shape, dtype))
        self.in_names = list(in_names)
        self.out_names = out_names
        self.out_avals = out_avals
        self.zero_outs = zero_outs
        n_params = len(in_names)
        all_in = in_names + out_names
        if partition_name is not None:
            all_in.append(partition_name)

        def _body(*args):
            operands = list(args)
            if partition_name is not None:
                operands.append(bass2jax.partition_id_tensor())
            return tuple(bass2jax._bass_exec_p.bind(
                *operands,
                out_avals=tuple(out_avals),
                in_names=tuple(all_in),
                out_names=tuple(out_names),
                lowering_input_output_aliases=(),
                sim_require_finite=True,
                sim_require_nnan=True,
                nc=nc,
            ))

        devices = jax.devices()[:NCORES]
        self.mesh = Mesh(np.asarray(devices), ("core",))
        self.sharding = NamedSharding(self.mesh, PartitionSpec("core"))
        n_in = n_params + len(out_names)
        self.sharded = jax.jit(shard_map(
            _body, mesh=self.mesh,
            in_specs=(PartitionSpec("core"),) * n_in,
            out_specs=(PartitionSpec("core"),) * len(out_names),
            check_rep=False), keep_unused=True)
        self._zeros_dev = None

    def warm(self):
        """trigger jit trace + neuronx compile with zero inputs."""
        zmap = {n: np.zeros(s, d) for n, (s, d) in self.in_specs.items()}
        self.run([zmap] * NCORES)

    def stage(self, in_maps):
        """host->device transfer of per-core inputs; returns device args."""
        jax = self.jax
        concat = [np.concatenate([np.asarray(m[n]) for m in in_maps], axis=0)
                  for n in self.in_names]
        args = [jax.device_put(a, self.sharding) for a in concat]
        if self._zeros_dev is None:
            self._zeros_dev = [
                jax.device_put(
                    np.zeros((NCORES * z.shape[0], *z.shape[1:]), z.dtype),
                    self.sharding) for z in self.zero_outs]
        args += self._zeros_dev
        for a in args:
            a.block_until_ready()
        return args

    def execute(self, args):
        outs = self.sharded(*args)
        for o in outs:
            o.block_until_ready()
        return outs

    def burst(self, args, reps=16, tries=3):
        """min total seconds for `reps` pipelined dispatches (async submit,
        block once at the end) — marginal per-exec isolates device time from
        the fixed dispatch floor."""
        import time as _t
        self.execute(args)  # warm
        best = float("inf")
        for _ in range(tries):
            t0 = _t.perf_counter()
            outs = None
            for _ in range(reps):
                outs = self.sharded(*args)
            for o in outs:
                o.block_until_ready()
            best = min(best, _t.perf_counter() - t0)
        return best / reps

    def run(self, in_maps, time_reps=0):
        args = self.stage(in_maps)
        outs = self.execute(args)  # cold (compiles first time)
        if time_reps:
            _run.times.append(int(self.burst(args) * 1e9))
        res = []
        for c in range(NCORES):
            res.append({
                name: np.asarray(outs[i]).reshape(
                    NCORES, *self.out_avals[i].shape)[c]
                for i, name in enumerate(self.out_names)})
        return res


import threading as _threading
_CACHE_LOCKS = {k: _threading.Lock() for k in (2, 8, "fc")}


def _get_nc(key):
    with _CACHE_LOCKS[key]:
        if key not in _CACHE:
            nc = build_fc() if key == "fc" else build_chain(key)
            _CACHE[key] = _Runner(nc)
    return _CACHE[key]


def _run(runner, in_maps, trace=False):
    return runner.run(in_maps, time_reps=3 if trace else 0)


_run.times = []


def _fc_shards(inp):
    Wfc = inp["Wfc"].astype(np.float32)
    bfc = inp["bfc"].astype(np.float32)
    shards = []
    for c in range(NCORES):
        v0 = c * VL
        wt = np.ascontiguousarray(
            Wfc[:, v0:v0 + VL].reshape(8, 128, VL).transpose(1, 0, 2)
        ).astype(nbf)
        bt = np.broadcast_to(bfc[v0:v0 + VL], (128, VL)).copy()
        shards.append((wt, bt))
    return shards


def kernel(**inputs):
    trace = bool(int(os.environ.get("CAPNET_TRACE", "0")))
    _run.times = []
    inp = {k: np.asarray(v) for k, v in inputs.items()}
    return _kernel_3phase(inp, trace)


def _kernel_3phase(inp, trace):
    per_dir = _chain_host_inputs(inp)

    # ---- phase 1: layer-0 chains (core 0 fwd, core 1 bwd)
    nc0 = _get_nc(2)
    maps0 = []
    for c in range(NCORES):
        d = c % 2
        m = {k: per_dir[d][k] for k in ("imgT", "Wh1t", "bh1t", "Wh2t", "bh2t",
                                        "Wc1t", "bc1t", "Wc2t", "bc2t")}
        m["Whh"] = per_dir[d]["Whh0"]
        m["Wih"] = per_dir[d]["Wih0"]
        m["bg"] = per_dir[d]["bg0"]
        maps0.append(m)
    x0f = _x0_arranged(inp, rev=False)
    x0b = _x0_arranged(inp, rev=True)
    for c in range(NCORES):
        maps0[c]["xt"] = x0f if c % 2 == 0 else x0b
    res0 = _run(nc0, maps0, trace=trace)
    h0f = _oh_to_HposB(res0[0]["oh"])
    h0b = _oh_to_HposB(res0[1]["oh"])

    # ---- phase 2: layer-1 chains
    nc1 = _get_nc(8)
    maps1 = []
    for c in range(NCORES):
        d = c % 2
        m = {k: per_dir[d][k] for k in ("imgT", "Wh1t", "bh1t", "Wh2t", "bh2t",
                                        "Wc1t", "bc1t", "Wc2t", "bc2t")}
        m["Whh"] = per_dir[d]["Whh1"]
        m["Wih"] = per_dir[d]["Wih1"]
        m["bg"] = per_dir[d]["bg1"]
        maps1.append(m)
    x1f = _x1_arranged(h0f, h0b, rev=False)
    x1b = _x1_arranged(h0f, h0b, rev=True)
    for c in range(NCORES):
        maps1[c]["xt"] = x1f if c % 2 == 0 else x1b
    res1 = _run(nc1, maps1, trace=trace)
    h1f = _oh_to_HposB(res1[0]["oh"])
    h1b = _oh_to_HposB(res1[1]["oh"])

    # ---- phase 3: FC head (vocab-sharded)
    ncf = _get_nc("fc")
    yT = _y_assemble(h1f, h1b)
    fcs = _fc_shards(inp)
    mapsf = [{"yT": yT, "Wfct": fcs[c][0], "bfcr": fcs[c][1]}
             for c in range(NCORES)]
    resf = _run(ncf, mapsf, trace=trace)

    logits = np.empty((N, T, B, V), np.float32)
    for c in range(NCORES):
        logits[:, :, :, c * VL:(c + 1) * VL] = (
            resf[c]["logits"][:800].reshape(N, T, B, VL))
    return logits
